# revision 20
# baseline (speedup 1.0000x reference)
"""BERT-base forward on 8 Trainium2 NeuronCores (Bass/Tile).

Strategy: data-parallel over batch (B=16 -> 2 per core) for the embedding +
12 transformer layers + pooler/cls + MLM gather/transform; the tied vocab
projection is sharded over the vocab axis in a second small SPMD launch
(each core: all 320 masked positions x 4000 vocab rows).

Layouts (per core, T = 2*512 = 1024 local tokens):
  - activations are feature-major: xT[p, k, t] = x[t, k*128+p]  (d = 6 tiles)
  - weights stay in natural [d_in, d_out] layout -> they are the PE lhsT
  - V is produced token-major for the attn@V matmul's lhsT
  - matmuls run in float32r (fp32 rounded to 11-bit mantissa, full PE speed)
"""
import sys
if '/opt/trn_rl_repo' not in sys.path:
    sys.path.insert(0, '/opt/trn_rl_repo')

import contextlib

import numpy as np

import concourse.bass as bass
import concourse.mybir as mybir
import concourse.tile as tile
from concourse import bacc, bass_utils, bass2jax

P = 128
D = 768
KD = 6            # d tiles
H = 12
DK = 64
DFF = 3072
KF = 24           # dff tiles
NL = 12
S = 512
BL = 2            # local batch
T = BL * S        # local tokens
NPRED = 20
V = 32000
VSH = 4000        # vocab shard per core
VSHP = 4096       # padded
NCORES = 8

F32 = mybir.dt.float32
F32R = mybir.dt.float32r
I32 = mybir.dt.int32
AF = mybir.ActivationFunctionType
ALU = mybir.AluOpType
AX = mybir.AxisListType


def round_fp32r(x: np.ndarray) -> np.ndarray:
    """Round fp32 to the fp32r grid (11-bit mantissa, RN-even)."""
    u = np.ascontiguousarray(x, dtype=np.float32).view(np.uint32)
    r = (u.astype(np.uint64) + 0x7FF + ((u >> 12) & 1)) & 0xFFFFF000
    return r.astype(np.uint32).view(np.float32)


# ---------------------------------------------------------------------------
# main program (embedding + 12 layers + pooler/cls + MLM transform)
# ---------------------------------------------------------------------------

def build_main(nl=NL, debug=False):
    nc = bacc.Bacc("TRN2", target_bir_lowering=False, debug=False,
                   num_devices=NCORES)

    # ---- inputs
    tok = nc.dram_tensor("tok", (V, D), F32, kind="ExternalInput")
    pos = nc.dram_tensor("pos", (S, D), F32, kind="ExternalInput")
    seg = nc.dram_tensor("seg", (2, D), F32, kind="ExternalInput")
    ids_idx = nc.dram_tensor("ids_idx", (P, 8), I32, kind="ExternalInput")
    seg_idx = nc.dram_tensor("seg_idx", (P, 8), I32, kind="ExternalInput")
    ssel_d = nc.dram_tensor("ssel", (P, 8, 40), F32R, kind="ExternalInput")
    emb_gb = nc.dram_tensor("emb_gb", (P, KD, 2), F32, kind="ExternalInput")
    mask01 = nc.dram_tensor("mask01", (P, BL, 4), F32, kind="ExternalInput")
    ident_d = nc.dram_tensor("ident", (P, P), F32R, kind="ExternalInput")
    ones512_d = nc.dram_tensor("ones512", (1, 512), F32R, kind="ExternalInput")
    onescol_d = nc.dram_tensor("onescol", (P, 1), F32R, kind="ExternalInput")
    eps_d = nc.dram_tensor("epsc", (P, 1), F32, kind="ExternalInput")

    Wqkv = nc.dram_tensor("Wqkv", (nl, D, 3 * D), F32R, kind="ExternalInput")
    bqk = nc.dram_tensor("bqk", (nl, P, 12), F32, kind="ExternalInput")
    ln1_d = nc.dram_tensor("ln1_gb", (nl, 2, D), F32R, kind="ExternalInput")
    ln2_d = nc.dram_tensor("ln2_gb", (nl, 2, D), F32R, kind="ExternalInput")
    # bias rows free-packed: [0:768]=bv, [768:1536]=bo, [1536:2304]=b2
    brow_d = nc.dram_tensor("brow", (nl, 1, 3 * D), F32R, kind="ExternalInput")
    Wo = nc.dram_tensor("Wo", (nl, D, D), F32R, kind="ExternalInput")
    W1 = nc.dram_tensor("W1", (nl, D, DFF), F32R, kind="ExternalInput")
    b1 = nc.dram_tensor("b1", (nl, P, KF), F32, kind="ExternalInput")
    W2 = nc.dram_tensor("W2", (nl, DFF, D), F32R, kind="ExternalInput")

    pool_W = nc.dram_tensor("pool_W", (D, D), F32R, kind="ExternalInput")
    pool_b = nc.dram_tensor("pool_b", (P, KD), F32, kind="ExternalInput")
    cls_W = nc.dram_tensor("cls_W", (D, 2), F32R, kind="ExternalInput")
    cls_b = nc.dram_tensor("cls_b", (2, 1), F32, kind="ExternalInput")
    lin_W = nc.dram_tensor("lin_W", (D, D), F32R, kind="ExternalInput")
    lin_b = nc.dram_tensor("lin_b", (P, KD), F32, kind="ExternalInput")

    # ---- outputs
    clsf_out = nc.dram_tensor("clsf_out", (2, BL), F32, kind="ExternalOutput")
    hm_out = nc.dram_tensor("hm_out", (D, 40), F32, kind="ExternalOutput")
    dbg = {}
    if debug:
        dbg['emb'] = nc.dram_tensor("dbg_emb", (D, T), F32, kind="ExternalOutput")
        dbg['hm_tm'] = nc.dram_tensor("dbg_hmtm", (P, D), F32, kind="ExternalOutput")
        dbg['x_tm'] = nc.dram_tensor("dbg_xtm", (T, D), F32, kind="ExternalOutput")
        for l in range(nl):
            dbg[f'x{l}'] = nc.dram_tensor(f"dbg_x{l}", (D, T), F32,
                                          kind="ExternalOutput")

    with tile.TileContext(nc) as tc:
        with contextlib.ExitStack() as ctx:
            sb = ctx.enter_context(tc.tile_pool(name="sb", bufs=1))
            sb2 = ctx.enter_context(tc.tile_pool(name="sb2", bufs=2))
            ps = ctx.enter_context(tc.tile_pool(name="ps", bufs=8, space="PSUM"))
            dramp = ctx.enter_context(tc.tile_pool(name="dram", bufs=1,
                                                   space="DRAM"))

            # ---------- persistent constants
            ident = sb.tile([P, P], F32R, tag="ident")
            nc.sync.dma_start(ident[:], ident_d[:, :])
            ones512 = sb.tile([1, 512], F32R, tag="ones512")
            nc.sync.dma_start(ones512[:], ones512_d[:, :])
            onescol = sb.tile([P, 1], F32R, tag="onescol")
            nc.sync.dma_start(onescol[:], onescol_d[:, :])
            epsc = sb.tile([P, 1], F32, tag="epsc")
            nc.sync.dma_start(epsc[:], eps_d[:, :])
            mask_sb = sb.tile([P, BL, 4], F32, tag="mask")
            nc.sync.dma_start(mask_sb[:], mask01[:, :, :])
            embgb = sb.tile([P, KD, 2], F32, tag="embgb")
            nc.sync.dma_start(embgb[:], emb_gb[:, :, :])

            # persistent activation buffer (updated in place across layers)
            xT = sb.tile([P, KD, T], F32R, tag="xT")

            # ---------- embedding (token-major), then transpose into xT
            idx_sb = sb.tile([P, 8], I32, tag="idx")
            nc.sync.dma_start(idx_sb[:], ids_idx[:, :])
            sidx_sb = sb.tile([P, 8], I32, tag="sidx")
            nc.sync.dma_start(sidx_sb[:], seg_idx[:, :])

            for tt in range(8):
                x0 = sb2.tile([P, D], F32, tag="wqk")
                nc.gpsimd.indirect_dma_start(
                    out=x0[:], out_offset=None, in_=tok[:],
                    in_offset=bass.IndirectOffsetOnAxis(ap=idx_sb[:, tt:tt + 1],
                                                        axis=0))
                nc.gpsimd.indirect_dma_start(
                    out=x0[:], out_offset=None, in_=seg[:],
                    in_offset=bass.IndirectOffsetOnAxis(ap=sidx_sb[:, tt:tt + 1],
                                                        axis=0),
                    compute_op=ALU.add)
                so = (tt % 4) * P
                nc.gpsimd.dma_start(x0[:], pos[so:so + P, :], accum_op=ALU.add)
                # LayerNorm over free dim (emb g/b applied after the transpose)
                sx = sb2.tile([P, 1], F32, tag="row_s")
                nc.vector.reduce_sum(sx[:], x0[:], axis=AX.X)
                sq = sb2.tile([P, D], F32, tag="w2c")
                nc.vector.tensor_tensor(sq[:], x0[:], x0[:], ALU.mult)
                sx2 = sb2.tile([P, 1], F32, tag="row_s2")
                nc.vector.reduce_sum(sx2[:], sq[:], axis=AX.X)
                m = sb2.tile([P, 1], F32, tag="row_m")
                nc.vector.tensor_scalar(m[:], sx[:], 1.0 / D, None, ALU.mult)
                ex2 = sb2.tile([P, 1], F32, tag="row_e")
                nc.vector.tensor_scalar(ex2[:], sx2[:], 1.0 / D, None, ALU.mult)
                var = sb2.tile([P, 1], F32, tag="row_v")
                nc.vector.tensor_tensor(var[:], m[:], m[:], ALU.mult)
                nc.vector.tensor_tensor(var[:], ex2[:], var[:], ALU.subtract)
                lnv = sb2.tile([P, 1], F32, tag="row_l")
                nc.scalar.activation(lnv[:], var[:], AF.Ln, bias=epsc[:, 0:1])
                rstd = sb2.tile([P, 1], F32, tag="row_r")
                nc.scalar.activation(rstd[:], lnv[:], AF.Exp, scale=-0.5)
                xc = sb2.tile([P, D], F32, tag="w2c")
                nc.vector.tensor_tensor(xc[:], x0[:],
                                        m[:].to_broadcast([P, D]), ALU.subtract)
                xh = sb2.tile([P, D], F32R, tag="w1c")
                nc.vector.tensor_tensor(xh[:], xc[:],
                                        rstd[:].to_broadcast([P, D]), ALU.mult)
                # transpose this token tile into feature-major, fusing emb g/b
                for kg in range(2):
                    nk = 4 if kg == 0 else 2
                    pT = ps.tile([P, 512], F32R, tag="ps")
                    for j in range(nk):
                        k = kg * 4 + j
                        nc.tensor.transpose(pT[:, j * P:(j + 1) * P],
                                            xh[:, k * P:(k + 1) * P], ident[:])
                    for j in range(nk):
                        k = kg * 4 + j
                        nc.scalar.activation(
                            xT[:, k, tt * P:(tt + 1) * P],
                            pT[:, j * P:(j + 1) * P], AF.Identity,
                            scale=embgb[:, k, 0:1], bias=embgb[:, k, 1:2])

            if debug:
                nc.sync.dma_start(dbg['emb'].rearrange("(k p) t -> p k t", p=P),
                                  xT[:].bitcast(F32))

            # ---------- transformer layers
            for l in range(nl):
                _emit_layer(nc, sb, sb2, ps, l, xT,
                            Wqkv, bqk, ln1_d, ln2_d, brow_d, Wo, W1, b1, W2,
                            ident, ones512_d, ones512, onescol, epsc, mask_sb)
                if debug:
                    nc.sync.dma_start(
                        dbg[f'x{l}'].rearrange("(k p) t -> p k t", p=P),
                        xT[:].bitcast(F32))

            # ---------- pooler + classifier
            hpT = sb.tile([P, KD, BL], F32R, tag="hpT")
            poolb_sb = sb.tile([P, KD], F32, tag="poolb")
            nc.sync.dma_start(poolb_sb[:], pool_b[:, :])
            x0T = xT[:, :, 0:S + 1:S]  # tokens 0 and 512 (CLS of both batches)
            for mt in range(KD):
                pwc = sb2.tile([P, KD, P], F32R, tag="w1c")
                nc.sync.dma_start(pwc[:], pool_W[:, mt * P:(mt + 1) * P]
                                  .rearrange("(ko p) n -> p ko n", p=P))
                pp = ps.tile([P, 512], F32, tag="ps")
                for k in range(KD):
                    nc.tensor.matmul(pp[:, :BL],
                                     pwc[:, k],
                                     x0T[:, k], start=(k == 0),
                                     stop=(k == KD - 1))
                nc.scalar.activation(hpT[:, mt], pp[:, :BL], AF.Tanh,
                                     bias=poolb_sb[:, mt:mt + 1])
            clsW_sb = sb.tile([P, KD, 2], F32R, tag="clsW")
            nc.sync.dma_start(clsW_sb[:],
                              cls_W.rearrange("(ko p) n -> p ko n", p=P))
            clsb_sb = sb.tile([2, 1], F32, tag="clsb")
            nc.sync.dma_start(clsb_sb[:], cls_b[:, :])
            pc = ps.tile([P, 512], F32, tag="ps")
            for k in range(KD):
                nc.tensor.matmul(pc[:2, :BL], clsW_sb[:, k], hpT[:, k],
                                 start=(k == 0), stop=(k == KD - 1))
            clsf_sb = sb.tile([2, BL], F32, tag="clsf")
            nc.scalar.activation(clsf_sb[:], pc[:2, :BL], AF.Identity,
                                 bias=clsb_sb[:, 0:1])
            nc.sync.dma_start(clsf_out[:, :], clsf_sb[:])

            # ---------- MLM: transpose final x, select masked rows via PE
            x_tm = sb.tile([P, 8, D], F32R, tag="qkT")
            for tt in range(8):
                for kg in range(2):
                    w = 512 if kg == 0 else 256
                    nk = 4 if kg == 0 else 2
                    pT = ps.tile([P, 512], F32R, tag="ps")
                    for j in range(nk):
                        k = kg * 4 + j
                        nc.tensor.transpose(pT[:, j * P:(j + 1) * P],
                                            xT[:, k, tt * P:(tt + 1) * P],
                                            ident[:])
                    nc.scalar.activation(x_tm[:, tt, kg * 512:kg * 512 + w],
                                         pT[:, :w], AF.Copy)
            ssel = sb.tile([P, 8, 40], F32R, tag="ssel")
            nc.sync.dma_start(ssel[:], ssel_d[:, :, :])
            # hmT[d, j] = sum_t x_tm[t, d] * ssel[t, j]
            hmT = sb.tile([P, KD, 40], F32R, tag="hmT")
            for k in range(KD):
                pT = ps.tile([P, 512], F32, tag="ps")
                for tt in range(8):
                    nc.tensor.matmul(pT[:, :40],
                                     x_tm[:, tt, k * P:(k + 1) * P],
                                     ssel[:, tt], start=(tt == 0),
                                     stop=(tt == 7))
                nc.scalar.activation(hmT[:, k], pT[:, :40], AF.Copy)
            linb_sb = sb.tile([P, KD], F32, tag="linb")
            nc.sync.dma_start(linb_sb[:], lin_b[:, :])
            hml = sb.tile([P, KD, 40], F32, tag="hml")
            for mt in range(KD):
                lwc = sb2.tile([P, KD, P], F32R, tag="w1c")
                nc.sync.dma_start(lwc[:], lin_W[:, mt * P:(mt + 1) * P]
                                  .rearrange("(ko p) n -> p ko n", p=P))
                pp = ps.tile([P, 512], F32, tag="ps")
                for k in range(KD):
                    nc.tensor.matmul(pp[:, :40],
                                     lwc[:, k],
                                     hmT[:, k], start=(k == 0),
                                     stop=(k == KD - 1))
                nc.scalar.activation(hml[:, mt], pp[:, :40], AF.Gelu,
                                     bias=linb_sb[:, mt:mt + 1])
            nc.sync.dma_start(hm_out.rearrange("(k p) j -> p k j", p=P), hml[:])

    nc.compile()
    return nc


def _emit_layer(nc, sb, sb2, ps, l, xT,
                Wqkv, bqk, ln1_d, ln2_d, brow_d, Wo, W1, b1, W2,
                ident, ones512_d, ones512, onescol, epsc, mask_sb):
    # ---- per-layer small loads
    bqk_sb = sb2.tile([P, 12], F32, tag="bqk")
    nc.sync.dma_start(bqk_sb[:], bqk[l, :, :])
    ln1r = sb.tile([2, D], F32R, tag="ln1r")
    nc.sync.dma_start(ln1r[:], ln1_d[l, :, :])
    ln2r = sb.tile([2, D], F32R, tag="ln2r")
    nc.sync.dma_start(ln2r[:], ln2_d[l, :, :])
    brow = sb.tile([1, 3 * D], F32R, tag="brow")
    nc.sync.dma_start(brow[:], brow_d[l, :, :])
    b1_sb = sb2.tile([P, KF], F32, tag="b1")
    nc.sync.dma_start(b1_sb[:], b1[l, :, :])
    bvr = brow[0:1, 0:D]
    bor = brow[0:1, D:2 * D]
    b2r = brow[0:1, 2 * D:3 * D]

    # Wv: full [768, 768] resident (rhs for V); Wo full (lhsT)
    xhat1 = sb.tile([P, KD, T], F32R, tag="xhat1")

    for b in range(BL):
        # ---------- Q/K projections for batch b (feature-major)
        qkT = sb.tile([P, 12, S], F32R, tag="qkT")
        for mg in range(6):  # m-groups of 2 of the 12 q/k out-tiles
            wqk = sb2.tile([P, KD, 2 * P], F32R, tag="wqk")
            nc.sync.dma_start(
                wqk[:], Wqkv[l, :, mg * 2 * P:(mg + 1) * 2 * P]
                .rearrange("(ko p) n -> p ko n", p=P))
            for mj in range(2):
                m = mg * 2 + mj  # 0-5 = q tiles, 6-11 = k tiles
                pqk = ps.tile([P, 512], F32, tag="ps")
                for k in range(KD):
                    nc.tensor.matmul(pqk[:], wqk[:, k, mj * P:(mj + 1) * P],
                                     xT[:, k, b * S:(b + 1) * S],
                                     start=(k == 0), stop=(k == KD - 1))
                nc.scalar.activation(qkT[:, m], pqk[:], AF.Identity,
                                     bias=bqk_sb[:, m:m + 1])

        # ---------- V projection for batch b (token-major)
        v_tm = sb.tile([P, 4, D], F32R, tag="v_tm")
        for ng in range(2):
            w = 512 if ng == 0 else 256
            wvc = sb.tile([P, KD, 512], F32R, tag="wvc")
            nc.sync.dma_start(wvc[:, :, :w],
                              Wqkv[l, :, 2 * D + ng * 512:2 * D + ng * 512 + w]
                              .rearrange("(ko p) n -> p ko n", p=P))
            for st in range(4):
                pv = ps.tile([P, 512], F32, tag="ps")
                for k in range(KD):
                    nc.tensor.matmul(pv[:, :w],
                                     xT[:, k, b * S + st * P:b * S + (st + 1) * P],
                                     wvc[:, k, :w],
                                     start=(k == 0), stop=False)
                nc.tensor.matmul(pv[:, :w], ones512[:, :P],
                                 bvr[:, ng * 512:ng * 512 + w],
                                 start=False, stop=True)
                nc.scalar.activation(v_tm[:, st, ng * 512:ng * 512 + w],
                                     pv[:, :w], AF.Copy)

        # ---------- attention heads
        ctxT = sb.tile([P, KD, S], F32R, tag="ctxT")
        for h in range(H):
            jq = h // 2
            pb = (h % 2) * DK
            probs = sb.tile([P, 4, S], F32R, tag="probs")
            for mt in range(4):
                psc = ps.tile([P, 512], F32, tag="ps")
                nc.tensor.matmul(psc[:],
                                 qkT[pb:pb + DK, jq, mt * P:(mt + 1) * P],
                                 qkT[pb:pb + DK, 6 + jq, :],
                                 start=True, stop=True)
                nc.scalar.activation(probs[:, mt], psc[:], AF.Exp, scale=0.125)
            probsT = sb.tile([P, 4, S], F32R, tag="probsT")
            for kt in range(4):
                pT = ps.tile([P, 512], F32R, tag="ps")
                for mt in range(4):
                    nc.tensor.transpose(pT[:, mt * P:(mt + 1) * P],
                                        probs[:, mt, kt * P:(kt + 1) * P],
                                        ident[:])
                # masked copy out of PSUM (pad keys -> 0), split DVE/ACT
                if kt % 2 == 0:
                    nc.vector.tensor_tensor(probsT[:, kt], pT[:],
                                            mask_sb[:, b, kt:kt + 1]
                                            .to_broadcast([P, S]), ALU.mult)
                else:
                    nc.scalar.activation(probsT[:, kt], pT[:], AF.Copy,
                                         scale=mask_sb[:, b, kt:kt + 1])
            # denominators: column sums of probsT via PE, then reciprocal
            psum_r = ps.tile([P, 512], F32, tag="ps")
            for kt in range(4):
                nc.tensor.matmul(psum_r[:1], onescol[:], probsT[:, kt],
                                 start=(kt == 0), stop=(kt == 3))
            recip = sb2.tile([1, S], F32, tag="recip")
            nc.vector.reciprocal(recip[:], psum_r[:1])
            recip_r = sb2.tile([1, S], F32R, tag="recipr")
            nc.vector.tensor_copy(recip_r[:], recip[:])
            prb = ps.tile([P, 512], F32, tag="ps")
            nc.tensor.matmul(prb[:DK], ones512[:, :DK], recip_r[:],
                             start=True, stop=True)
            rb_sb = sb2.tile([DK, S], F32, tag="lnt1")
            nc.scalar.activation(rb_sb[:], prb[:DK], AF.Copy)
            # ctx.T[dv, tq] = sum_tk v[tk, dv] * probsT[tk, tq], then normalize
            pctx = ps.tile([P, 512], F32, tag="ps")
            for kt in range(4):
                nc.tensor.matmul(pctx[:DK], v_tm[:, kt, h * DK:(h + 1) * DK],
                                 probsT[:, kt], start=(kt == 0), stop=(kt == 3))
            nc.vector.tensor_tensor(ctxT[pb:pb + DK, jq], pctx[:DK], rb_sb[:],
                                    ALU.mult)

        # ---------- attention out projection + residual + LN1 for batch b
        h1 = sb.tile([P, KD, S], F32R, tag="v_tm")
        for m in range(KD):
            woch = sb2.tile([P, KD, P], F32R, tag="w1c")
            nc.sync.dma_start(woch[:], Wo[l, :, m * P:(m + 1) * P]
                              .rearrange("(ko p) n -> p ko n", p=P))
            po = ps.tile([P, 512], F32, tag="ps")
            for k in range(KD):
                nc.tensor.matmul(po[:], woch[:, k],
                                 ctxT[:, k], start=(k == 0), stop=False)
            nc.tensor.matmul(po[:], bor[:, m * P:(m + 1) * P], ones512[:],
                             start=False, stop=True)
            nc.vector.tensor_tensor(h1[:, m], po[:],
                                    xT[:, m, b * S:(b + 1) * S].bitcast(F32),
                                    ALU.add)
        _emit_ln(nc, sb, sb2, ps, h1, xhat1[:, :, b * S:(b + 1) * S], ln1r,
                 ones512_d, onescol, epsc)

    # ---------- feed-forward (fused FF1->FF2 per 512-token half) + LN2
    for b in range(BL):
        pf2 = [ps.tile([P, 512], F32, tag="ps", name=f"pf2_{m}")
               for m in range(KD)]
        for kk in range(KF):
            w1c = sb2.tile([P, KD, P], F32R, tag="w1c")
            nc.sync.dma_start(w1c[:], W1[l, :, kk * P:(kk + 1) * P]
                              .rearrange("(ko p) n -> p ko n", p=P))
            w2c = sb2.tile([P, D], F32R, tag="w2c")
            nc.sync.dma_start(w2c[:], W2[l, kk * P:(kk + 1) * P, :])
            pf1 = ps.tile([P, 512], F32, tag="ps")
            for k in range(KD):
                nc.tensor.matmul(pf1[:], w1c[:, k],
                                 xhat1[:, k, b * S:(b + 1) * S],
                                 start=(k == 0), stop=(k == KD - 1))
            f1 = sb2.tile([P, 512], F32R, tag="f1")
            nc.scalar.activation(f1[:], pf1[:], AF.Gelu,
                                 bias=b1_sb[:, kk:kk + 1])
            for m in range(KD):
                nc.tensor.matmul(pf2[m][:], w2c[:, m * P:(m + 1) * P], f1[:],
                                 start=(kk == 0), stop=False)
        h2 = sb.tile([P, KD, S], F32R, tag="v_tm")
        for m in range(KD):
            nc.tensor.matmul(pf2[m][:], b2r[:, m * P:(m + 1) * P], ones512[:],
                             start=False, stop=True)
            nc.vector.tensor_tensor(h2[:, m], pf2[m][:],
                                    xhat1[:, m, b * S:(b + 1) * S].bitcast(F32),
                                    ALU.add)
        _emit_ln(nc, sb, sb2, ps, h2, xT[:, :, b * S:(b + 1) * S], ln2r,
                 ones512_d, onescol, epsc)


def _emit_ln(nc, sb, sb2, ps, hin, xout, gbT, ones512_d, onescol, epsc):
    """LayerNorm over features (partition dim across KD tiles) of hin
    [P, KD, S] (F32R), writing g*(h-m)/sd + b into xout [P, KD, S]."""
    psx = ps.tile([P, 512], F32, tag="ps")
    for k in range(KD):
        nc.tensor.matmul(psx[:1], onescol[:], hin[:, k],
                         start=(k == 0), stop=(k == KD - 1))
    psx2 = ps.tile([P, 512], F32, tag="ps")
    for k in range(KD):
        sq = sb2.tile([P, 512], F32R, tag="f1")
        nc.scalar.activation(sq[:], hin[:, k], AF.Square)
        nc.tensor.matmul(psx2[:1], onescol[:], sq[:],
                         start=(k == 0), stop=(k == KD - 1))
    rowA = sb2.tile([1, S], F32, tag="lnA")   # m
    nc.vector.tensor_scalar(rowA[:], psx[:1], 1.0 / D, None, ALU.mult)
    rowB = sb2.tile([1, S], F32, tag="lnB")   # ex2 -> var -> rstd
    nc.vector.tensor_scalar(rowB[:], psx2[:1], 1.0 / D, None, ALU.mult)
    rowC = sb2.tile([1, S], F32, tag="lnC")   # m*m -> lnv -> mr
    nc.vector.tensor_tensor(rowC[:], rowA[:], rowA[:], ALU.mult)
    nc.vector.tensor_tensor(rowB[:], rowB[:], rowC[:], ALU.subtract)
    nc.scalar.activation(rowC[:], rowB[:], AF.Ln, bias=epsc[0:1, 0:1])
    rowR = sb.tile([1, S], F32R, tag="lnR")  # rstd; sole writer is ACT (f32r)
    nc.scalar.activation(rowR[:], rowC[:], AF.Exp, scale=-0.5)
    nc.vector.tensor_tensor(rowC[:], rowA[:], rowR[:].bitcast(F32),
                            ALU.mult)  # m*rstd
    rstd_r = rowR[:]
    # rows2 = [c ; ones] for the K=2 C' broadcast matmul
    rows2 = sb2.tile([2, S], F32R, tag="rows2")
    nc.scalar.activation(rows2[0:1], rowC[:], AF.Copy, scale=-1.0)
    nc.sync.dma_start(rows2[1:2], ones512_d[:, :])
    for k in range(KD):
        pA = ps.tile([P, 512], F32, tag="ps")
        nc.tensor.matmul(pA[:], gbT[0:1, k * P:(k + 1) * P], rstd_r,
                         start=True, stop=True)
        pC = ps.tile([P, 512], F32, tag="ps")
        nc.tensor.matmul(pC[:], gbT[:, k * P:(k + 1) * P], rows2[:],
                         start=True, stop=True)
        t1 = sb2.tile([P, 512], F32, tag="lnt1")
        nc.vector.tensor_tensor(t1[:], hin[:, k].bitcast(F32), pA[:], ALU.mult)
        nc.vector.tensor_tensor(xout[:, k], t1[:], pC[:], ALU.add)


# ---------------------------------------------------------------------------
# vocab-projection program (launch B): logits = h_all @ tokT_shard
# ---------------------------------------------------------------------------

def build_vocab():
    nc = bacc.Bacc("TRN2", target_bir_lowering=False, debug=False,
                   num_devices=NCORES)
    hallT = nc.dram_tensor("hallT", (D, 320), F32R, kind="ExternalInput")
    tokT = nc.dram_tensor("tokT", (D, VSHP), F32R, kind="ExternalInput")
    lm = nc.dram_tensor("lm", (320, VSHP), F32, kind="ExternalOutput")
    with tile.TileContext(nc) as tc:
        with tc.tile_pool(name="sb", bufs=1) as sb, \
             tc.tile_pool(name="sb2", bufs=3) as sb2, \
             tc.tile_pool(name="ps", bufs=8, space="PSUM") as ps:
            hall = sb.tile([P, KD, 320], F32R, tag="hall")
            nc.sync.dma_start(hall[:],
                              hallT.rearrange("(ko p) t -> p ko t", p=P))
            for nv in range(VSHP // 512):
                tc_sb = sb2.tile([P, KD, 512], F32R, tag="tokc")
                nc.sync.dma_start(tc_sb[:], tokT[:, nv * 512:(nv + 1) * 512]
                                  .rearrange("(ko p) n -> p ko n", p=P))
                for mt in range(3):
                    mw = 128 if mt < 2 else 64
                    pp = ps.tile([P, 512], F32, tag="ps")
                    for k in range(KD):
                        nc.tensor.matmul(pp[:mw],
                                         hall[:, k, mt * P:mt * P + mw],
                                         tc_sb[:, k], start=(k == 0),
                                         stop=(k == KD - 1))
                    ot = sb2.tile([P, 512], F32, tag="ot")
                    nc.scalar.activation(ot[:mw], pp[:mw], AF.Copy)
                    nc.sync.dma_start(lm[mt * P:mt * P + mw,
                                         nv * 512:(nv + 1) * 512], ot[:mw])
    nc.compile()
    return nc


# ---------------------------------------------------------------------------
# cached PJRT runner (compile once, reuse executable + device inputs)
# ---------------------------------------------------------------------------

class Runner:
    def __init__(self, nc, n_cores=NCORES):
        import jax
        from jax.sharding import Mesh, PartitionSpec
        from jax.experimental.shard_map import shard_map
        bass2jax.install_neuronx_cc_hook()
        self.nc = nc
        self.n_cores = n_cores
        partition_name = (nc.partition_id_tensor.name
                          if nc.partition_id_tensor else None)
        in_names, out_names, out_avals, zero_shapes = [], [], [], []
        for alloc in nc.m.functions[0].allocations:
            if not isinstance(alloc, mybir.MemoryLocationSet):
                continue
            name = alloc.memorylocations[0].name
            if alloc.kind == "ExternalInput":
                if name != partition_name:
                    in_names.append(name)
            elif alloc.kind == "ExternalOutput":
                shape = tuple(alloc.tensor_shape)
                dtype = mybir.dt.np(alloc.dtype)
                out_names.append(name)
                out_avals.append(jax.core.ShapedArray(shape, dtype))
                zero_shapes.append((shape, dtype))
        self.n_params = len(in_names)
        self.in_names = list(in_names)
        self.out_names = out_names
        self.out_avals = out_avals
        self.zero_shapes = zero_shapes
        all_in = in_names + out_names
        if partition_name is not None:
            all_in = all_in + [partition_name]

        def _body(*args):
            operands = list(args)
            if partition_name is not None:
                operands.append(bass2jax.partition_id_tensor())
            outs = bass2jax._bass_exec_p.bind(
                *operands,
                out_avals=tuple(out_avals),
                in_names=tuple(all_in),
                out_names=tuple(out_names),
                lowering_input_output_aliases=(),
                sim_require_finite=True,
                sim_require_nnan=True,
                nc=nc,
            )
            return tuple(outs)

        devices = jax.devices()[:n_cores]
        self.mesh = Mesh(np.asarray(devices), ("core",))
        n_outs = len(out_names)
        donate = tuple(range(self.n_params, self.n_params + n_outs))
        self.fn = jax.jit(
            shard_map(_body, mesh=self.mesh,
                      in_specs=(PartitionSpec("core"),) * (self.n_params + n_outs),
                      out_specs=(PartitionSpec("core"),) * n_outs,
                      check_rep=False),
            donate_argnums=donate, keep_unused=True)

    def put_inputs(self, in_maps):
        import jax
        from jax.sharding import NamedSharding, PartitionSpec
        sh = NamedSharding(self.mesh, PartitionSpec("core"))
        out = []
        for name in self.in_names:
            a = np.concatenate([np.asarray(m[name]) for m in in_maps], axis=0)
            out.append(jax.device_put(a, sh))
        return out

    def zeros(self):
        return [np.zeros((self.n_cores * s[0], *s[1:]), d)
                for (s, d) in self.zero_shapes]

    def run(self, dev_in):
        import jax
        outs = self.fn(*dev_in, *self.zeros())
        jax.block_until_ready(outs)
        return outs

    def split(self, out_arrs):
        res = []
        for c in range(self.n_cores):
            res.append({name: np.asarray(out_arrs[i])
                        .reshape(self.n_cores, *self.out_avals[i].shape)[c]
                        for i, name in enumerate(self.out_names)})
        return res

    def __call__(self, in_maps):
        return self.split(self.run(self.put_inputs(in_maps)))


# ---------------------------------------------------------------------------
# host-side preparation + execution
# ---------------------------------------------------------------------------

_CACHE = {}


def _prep_shared(params, nl=NL):
    """Build the shared (non-per-core) input arrays from params."""
    Lp = params['layers']
    g = {}
    tok = np.ascontiguousarray(np.asarray(params['tok'], dtype=np.float32))
    g['tok'] = tok
    g['pos'] = np.ascontiguousarray(np.asarray(params['pos'], np.float32)[:S])
    g['seg'] = np.ascontiguousarray(np.asarray(params['seg'], np.float32))
    eg = np.asarray(params['emb_g'], np.float32)
    eb = np.asarray(params['emb_b'], np.float32)
    g['emb_gb'] = np.ascontiguousarray(
        np.stack([eg.reshape(KD, P).T, eb.reshape(KD, P).T], axis=2))
    g['ident'] = round_fp32r(np.eye(P, dtype=np.float32))
    g['ones512'] = round_fp32r(np.ones((1, 512), np.float32))
    g['onescol'] = round_fp32r(np.ones((P, 1), np.float32))
    g['epsc'] = np.full((P, 1), 1e-5, np.float32)

    Wq = np.asarray(Lp['Wq'], np.float32)[:nl]
    Wk = np.asarray(Lp['Wk'], np.float32)[:nl]
    Wv = np.asarray(Lp['Wv'], np.float32)[:nl]
    g['Wqkv'] = round_fp32r(np.concatenate([Wq, Wk, Wv], axis=2))
    bq = np.asarray(Lp['bq'], np.float32)[:nl].reshape(nl, KD, P)
    bk = np.asarray(Lp['bk'], np.float32)[:nl].reshape(nl, KD, P)
    g['bqk'] = np.ascontiguousarray(
        np.concatenate([bq, bk], axis=1).transpose(0, 2, 1))
    g['ln1_gb'] = round_fp32r(np.stack(
        [np.asarray(Lp['ln1_g'], np.float32)[:nl],
         np.asarray(Lp['ln1_b'], np.float32)[:nl]], axis=1))
    g['ln2_gb'] = round_fp32r(np.stack(
        [np.asarray(Lp['ln2_g'], np.float32)[:nl],
         np.asarray(Lp['ln2_b'], np.float32)[:nl]], axis=1))
    g['brow'] = round_fp32r(np.concatenate(
        [np.asarray(Lp['bv'], np.float32)[:nl],
         np.asarray(Lp['bo'], np.float32)[:nl],
         np.asarray(Lp['b2'], np.float32)[:nl]], axis=1)[:, None, :])
    g['Wo'] = round_fp32r(np.asarray(Lp['Wo'], np.float32)[:nl])
    g['W1'] = round_fp32r(np.asarray(Lp['W1'], np.float32)[:nl])
    g['b1'] = np.ascontiguousarray(
        np.asarray(Lp['b1'], np.float32)[:nl].reshape(nl, KF, P)
        .transpose(0, 2, 1))
    g['W2'] = round_fp32r(np.asarray(Lp['W2'], np.float32)[:nl])
    g['pool_W'] = round_fp32r(np.asarray(params['pool_W'], np.float32))
    g['pool_b'] = np.ascontiguousarray(
        np.asarray(params['pool_b'], np.float32).reshape(KD, P).T)
    g['cls_W'] = round_fp32r(np.asarray(params['cls_W'], np.float32))
    g['cls_b'] = np.asarray(params['cls_b'], np.float32).reshape(2, 1)
    g['lin_W'] = round_fp32r(np.asarray(params['lin_W'], np.float32))
    g['lin_b'] = np.ascontiguousarray(
        np.asarray(params['lin_b'], np.float32).reshape(KD, P).T)
    return g


def make_in_maps(ids, segs, mp, g):
    in_maps = []
    for c in range(NCORES):
        im = dict(g)
        cid = ids[c * BL:(c + 1) * BL]
        cseg = segs[c * BL:(c + 1) * BL]
        cmp = mp[c * BL:(c + 1) * BL]
        im['ids_idx'] = np.ascontiguousarray(cid.reshape(8, P).T)
        im['seg_idx'] = np.ascontiguousarray(cseg.reshape(8, P).T)
        mpg = np.concatenate([cmp[0], cmp[1] + S]).astype(np.int64)
        ssel = np.zeros((P, 8, 40), np.float32)
        for j, t in enumerate(mpg):
            ssel[t % P, t // P, j] = 1.0
        im['ssel'] = ssel
        m01 = (cid != 0).astype(np.float32)
        im['mask01'] = np.ascontiguousarray(
            m01.reshape(BL, 4, P).transpose(2, 0, 1))
        in_maps.append(im)
    return in_maps


def kernel(input_ids, segment_ids, masked_pos, params):
    ids = np.asarray(input_ids).astype(np.int32)
    segs = np.asarray(segment_ids).astype(np.int32)
    mp = np.asarray(masked_pos).astype(np.int32)
    B = ids.shape[0]
    assert B == NCORES * BL

    if 'main_r' not in _CACHE:
        _CACHE['main_r'] = Runner(build_main())
    if 'vocab_r' not in _CACHE:
        _CACHE['vocab_r'] = Runner(build_vocab())

    g = _prep_shared(params)
    in_maps = make_in_maps(ids, segs, mp, g)

    results = _CACHE['main_r'](in_maps)
    hm = np.stack([results[c]['hm_out'] for c in range(NCORES)])
    clsf = np.concatenate([results[c]['clsf_out'].T
                           for c in range(NCORES)])

    # vocab projection launch (vocab-sharded over the 8 cores)
    hallT = round_fp32r(np.concatenate(list(hm), axis=1))  # [768, 320]
    in_maps2 = []
    for c in range(NCORES):
        tksh = np.zeros((D, VSHP), np.float32)
        tksh[:, :VSH] = g['tok'][c * VSH:(c + 1) * VSH, :].T
        in_maps2.append({'hallT': hallT, 'tokT': round_fp32r(tksh)})
    results2 = _CACHE['vocab_r'](in_maps2)
    lm = np.concatenate([results2[c]['lm'][:, :VSH]
                         for c in range(NCORES)], axis=1)
    logits_lm = lm.reshape(B, NPRED, V)
    return logits_lm, clsf


# revision 21
# speedup vs baseline: 23.8784x; 23.8784x over previous
"""BERT-base forward on 8 Trainium2 NeuronCores (Bass/Tile).

Strategy: data-parallel over batch (B=16 -> 2 per core) for the embedding +
12 transformer layers + pooler/cls + MLM gather/transform; the tied vocab
projection is sharded over the vocab axis in a second small SPMD launch
(each core: all 320 masked positions x 4000 vocab rows).

Layouts (per core, T = 2*512 = 1024 local tokens):
  - activations are feature-major: xT[p, k, t] = x[t, k*128+p]  (d = 6 tiles)
  - weights stay in natural [d_in, d_out] layout -> they are the PE lhsT
  - V is produced token-major for the attn@V matmul's lhsT
  - matmuls run in float32r (fp32 rounded to 11-bit mantissa, full PE speed)
"""
import sys
if '/opt/trn_rl_repo' not in sys.path:
    sys.path.insert(0, '/opt/trn_rl_repo')

import contextlib

import numpy as np

import concourse.bass as bass
import concourse.mybir as mybir
import concourse.tile as tile
from concourse import bacc, bass_utils, bass2jax

P = 128
D = 768
KD = 6            # d tiles
H = 12
DK = 64
DFF = 3072
KF = 24           # dff tiles
NL = 12
S = 512
BL = 2            # local batch
T = BL * S        # local tokens
NPRED = 20
V = 32000
VSH = 4000        # vocab shard per core
VSHP = 4096       # padded
NCORES = 8

F32 = mybir.dt.float32
F32R = mybir.dt.float32r
I32 = mybir.dt.int32
AF = mybir.ActivationFunctionType
ALU = mybir.AluOpType
AX = mybir.AxisListType


def round_fp32r(x: np.ndarray) -> np.ndarray:
    """Round fp32 to the fp32r grid (11-bit mantissa, RN-even)."""
    u = np.ascontiguousarray(x, dtype=np.float32).view(np.uint32)
    r = (u.astype(np.uint64) + 0x7FF + ((u >> 12) & 1)) & 0xFFFFF000
    return r.astype(np.uint32).view(np.float32)


# ---------------------------------------------------------------------------
# main program (embedding + 12 layers + pooler/cls + MLM transform)
# ---------------------------------------------------------------------------

def build_main(nl=NL, debug=False):
    nc = bacc.Bacc("TRN2", target_bir_lowering=False, debug=False,
                   num_devices=NCORES)

    # ---- inputs
    tok = nc.dram_tensor("tok", (V, D), F32, kind="ExternalInput")
    pos = nc.dram_tensor("pos", (S, D), F32, kind="ExternalInput")
    seg = nc.dram_tensor("seg", (2, D), F32, kind="ExternalInput")
    ids_idx = nc.dram_tensor("ids_idx", (P, 8), I32, kind="ExternalInput")
    seg_idx = nc.dram_tensor("seg_idx", (P, 8), I32, kind="ExternalInput")
    ssel_d = nc.dram_tensor("ssel", (P, 8, 40), F32R, kind="ExternalInput")
    emb_gb = nc.dram_tensor("emb_gb", (P, KD, 2), F32, kind="ExternalInput")
    mask01 = nc.dram_tensor("mask01", (P, BL, 4), F32, kind="ExternalInput")
    ident_d = nc.dram_tensor("ident", (P, P), F32R, kind="ExternalInput")
    ones512_d = nc.dram_tensor("ones512", (1, 512), F32R, kind="ExternalInput")
    onescol_d = nc.dram_tensor("onescol", (P, 1), F32R, kind="ExternalInput")
    eps_d = nc.dram_tensor("epsc", (P, 1), F32, kind="ExternalInput")

    Wqkv = nc.dram_tensor("Wqkv", (nl, D, 3 * D), F32R, kind="ExternalInput")
    bqk = nc.dram_tensor("bqk", (nl, P, 12), F32, kind="ExternalInput")
    ln1_d = nc.dram_tensor("ln1_gb", (nl, 2, D), F32R, kind="ExternalInput")
    ln2_d = nc.dram_tensor("ln2_gb", (nl, 2, D), F32R, kind="ExternalInput")
    # bias rows free-packed: [0:768]=bv, [768:1536]=bo, [1536:2304]=b2
    brow_d = nc.dram_tensor("brow", (nl, 1, 3 * D), F32R, kind="ExternalInput")
    Wo = nc.dram_tensor("Wo", (nl, D, D), F32R, kind="ExternalInput")
    W1 = nc.dram_tensor("W1", (nl, D, DFF), F32R, kind="ExternalInput")
    b1 = nc.dram_tensor("b1", (nl, P, KF), F32, kind="ExternalInput")
    W2 = nc.dram_tensor("W2", (nl, DFF, D), F32R, kind="ExternalInput")

    pool_W = nc.dram_tensor("pool_W", (D, D), F32R, kind="ExternalInput")
    pool_b = nc.dram_tensor("pool_b", (P, KD), F32, kind="ExternalInput")
    cls_W = nc.dram_tensor("cls_W", (D, 2), F32R, kind="ExternalInput")
    cls_b = nc.dram_tensor("cls_b", (2, 1), F32, kind="ExternalInput")
    lin_W = nc.dram_tensor("lin_W", (D, D), F32R, kind="ExternalInput")
    lin_b = nc.dram_tensor("lin_b", (P, KD), F32, kind="ExternalInput")

    # ---- outputs
    clsf_out = nc.dram_tensor("clsf_out", (2, BL), F32, kind="ExternalOutput")
    hm_out = nc.dram_tensor("hm_out", (D, 40), F32, kind="ExternalOutput")
    dbg = {}
    if debug:
        dbg['emb'] = nc.dram_tensor("dbg_emb", (D, T), F32, kind="ExternalOutput")
        dbg['hm_tm'] = nc.dram_tensor("dbg_hmtm", (P, D), F32, kind="ExternalOutput")
        dbg['x_tm'] = nc.dram_tensor("dbg_xtm", (T, D), F32, kind="ExternalOutput")
        for l in range(nl):
            dbg[f'x{l}'] = nc.dram_tensor(f"dbg_x{l}", (D, T), F32,
                                          kind="ExternalOutput")

    with tile.TileContext(nc) as tc:
        with contextlib.ExitStack() as ctx:
            sb = ctx.enter_context(tc.tile_pool(name="sb", bufs=1))
            sb2 = ctx.enter_context(tc.tile_pool(name="sb2", bufs=2))
            ps = ctx.enter_context(tc.tile_pool(name="ps", bufs=8, space="PSUM"))
            dramp = ctx.enter_context(tc.tile_pool(name="dram", bufs=1,
                                                   space="DRAM"))

            # ---------- persistent constants
            ident = sb.tile([P, P], F32R, tag="ident")
            nc.sync.dma_start(ident[:], ident_d[:, :])
            ones512 = sb.tile([1, 512], F32R, tag="ones512")
            nc.sync.dma_start(ones512[:], ones512_d[:, :])
            onescol = sb.tile([P, 1], F32R, tag="onescol")
            nc.sync.dma_start(onescol[:], onescol_d[:, :])
            epsc = sb.tile([P, 1], F32, tag="epsc")
            nc.sync.dma_start(epsc[:], eps_d[:, :])
            mask_sb = sb.tile([P, BL, 4], F32, tag="mask")
            nc.sync.dma_start(mask_sb[:], mask01[:, :, :])
            embgb = sb.tile([P, KD, 2], F32, tag="embgb")
            nc.sync.dma_start(embgb[:], emb_gb[:, :, :])

            # persistent activation buffer (updated in place across layers)
            xT = sb.tile([P, KD, T], F32R, tag="xT")

            # ---------- embedding (token-major), then transpose into xT
            idx_sb = sb.tile([P, 8], I32, tag="idx")
            nc.sync.dma_start(idx_sb[:], ids_idx[:, :])
            sidx_sb = sb.tile([P, 8], I32, tag="sidx")
            nc.sync.dma_start(sidx_sb[:], seg_idx[:, :])

            for tt in range(8):
                x0 = sb2.tile([P, D], F32, tag="wqk")
                nc.gpsimd.indirect_dma_start(
                    out=x0[:], out_offset=None, in_=tok[:],
                    in_offset=bass.IndirectOffsetOnAxis(ap=idx_sb[:, tt:tt + 1],
                                                        axis=0))
                nc.gpsimd.indirect_dma_start(
                    out=x0[:], out_offset=None, in_=seg[:],
                    in_offset=bass.IndirectOffsetOnAxis(ap=sidx_sb[:, tt:tt + 1],
                                                        axis=0),
                    compute_op=ALU.add)
                so = (tt % 4) * P
                nc.gpsimd.dma_start(x0[:], pos[so:so + P, :], accum_op=ALU.add)
                # LayerNorm over free dim (emb g/b applied after the transpose)
                sx = sb2.tile([P, 1], F32, tag="row_s")
                nc.vector.reduce_sum(sx[:], x0[:], axis=AX.X)
                sq = sb2.tile([P, D], F32, tag="w2c")
                nc.vector.tensor_tensor(sq[:], x0[:], x0[:], ALU.mult)
                sx2 = sb2.tile([P, 1], F32, tag="row_s2")
                nc.vector.reduce_sum(sx2[:], sq[:], axis=AX.X)
                m = sb2.tile([P, 1], F32, tag="row_m")
                nc.vector.tensor_scalar(m[:], sx[:], 1.0 / D, None, ALU.mult)
                ex2 = sb2.tile([P, 1], F32, tag="row_e")
                nc.vector.tensor_scalar(ex2[:], sx2[:], 1.0 / D, None, ALU.mult)
                var = sb2.tile([P, 1], F32, tag="row_v")
                nc.vector.tensor_tensor(var[:], m[:], m[:], ALU.mult)
                nc.vector.tensor_tensor(var[:], ex2[:], var[:], ALU.subtract)
                lnv = sb2.tile([P, 1], F32, tag="row_l")
                nc.scalar.activation(lnv[:], var[:], AF.Ln, bias=epsc[:, 0:1])
                rstd = sb2.tile([P, 1], F32, tag="row_r")
                nc.scalar.activation(rstd[:], lnv[:], AF.Exp, scale=-0.5)
                xc = sb2.tile([P, D], F32, tag="w2c")
                nc.vector.tensor_tensor(xc[:], x0[:],
                                        m[:].to_broadcast([P, D]), ALU.subtract)
                xh = sb2.tile([P, D], F32R, tag="w1c")
                nc.vector.tensor_tensor(xh[:], xc[:],
                                        rstd[:].to_broadcast([P, D]), ALU.mult)
                # transpose this token tile into feature-major, fusing emb g/b
                for kg in range(2):
                    nk = 4 if kg == 0 else 2
                    pT = ps.tile([P, 512], F32R, tag="ps")
                    for j in range(nk):
                        k = kg * 4 + j
                        nc.tensor.transpose(pT[:, j * P:(j + 1) * P],
                                            xh[:, k * P:(k + 1) * P], ident[:])
                    for j in range(nk):
                        k = kg * 4 + j
                        nc.scalar.activation(
                            xT[:, k, tt * P:(tt + 1) * P],
                            pT[:, j * P:(j + 1) * P], AF.Identity,
                            scale=embgb[:, k, 0:1], bias=embgb[:, k, 1:2])

            if debug:
                nc.sync.dma_start(dbg['emb'].rearrange("(k p) t -> p k t", p=P),
                                  xT[:].bitcast(F32))

            # ---------- transformer layers
            for l in range(nl):
                _emit_layer(nc, sb, sb2, ps, l, xT,
                            Wqkv, bqk, ln1_d, ln2_d, brow_d, Wo, W1, b1, W2,
                            ident, ones512_d, ones512, onescol, epsc, mask_sb)
                if debug:
                    nc.sync.dma_start(
                        dbg[f'x{l}'].rearrange("(k p) t -> p k t", p=P),
                        xT[:].bitcast(F32))

            # ---------- pooler + classifier
            hpT = sb.tile([P, KD, BL], F32R, tag="hpT")
            poolb_sb = sb.tile([P, KD], F32, tag="poolb")
            nc.sync.dma_start(poolb_sb[:], pool_b[:, :])
            x0T = xT[:, :, 0:S + 1:S]  # tokens 0 and 512 (CLS of both batches)
            for mt in range(KD):
                pwc = sb2.tile([P, KD, P], F32R, tag="w1c")
                nc.sync.dma_start(pwc[:], pool_W[:, mt * P:(mt + 1) * P]
                                  .rearrange("(ko p) n -> p ko n", p=P))
                pp = ps.tile([P, 512], F32, tag="ps")
                for k in range(KD):
                    nc.tensor.matmul(pp[:, :BL],
                                     pwc[:, k],
                                     x0T[:, k], start=(k == 0),
                                     stop=(k == KD - 1))
                nc.scalar.activation(hpT[:, mt], pp[:, :BL], AF.Tanh,
                                     bias=poolb_sb[:, mt:mt + 1])
            clsW_sb = sb.tile([P, KD, 2], F32R, tag="clsW")
            nc.sync.dma_start(clsW_sb[:],
                              cls_W.rearrange("(ko p) n -> p ko n", p=P))
            clsb_sb = sb.tile([2, 1], F32, tag="clsb")
            nc.sync.dma_start(clsb_sb[:], cls_b[:, :])
            pc = ps.tile([P, 512], F32, tag="ps")
            for k in range(KD):
                nc.tensor.matmul(pc[:2, :BL], clsW_sb[:, k], hpT[:, k],
                                 start=(k == 0), stop=(k == KD - 1))
            clsf_sb = sb.tile([2, BL], F32, tag="clsf")
            nc.scalar.activation(clsf_sb[:], pc[:2, :BL], AF.Identity,
                                 bias=clsb_sb[:, 0:1])
            nc.sync.dma_start(clsf_out[:, :], clsf_sb[:])

            # ---------- MLM: transpose final x, select masked rows via PE
            x_tm = sb.tile([P, 8, D], F32R, tag="qkT")
            for tt in range(8):
                for kg in range(2):
                    w = 512 if kg == 0 else 256
                    nk = 4 if kg == 0 else 2
                    pT = ps.tile([P, 512], F32R, tag="ps")
                    for j in range(nk):
                        k = kg * 4 + j
                        nc.tensor.transpose(pT[:, j * P:(j + 1) * P],
                                            xT[:, k, tt * P:(tt + 1) * P],
                                            ident[:])
                    nc.scalar.activation(x_tm[:, tt, kg * 512:kg * 512 + w],
                                         pT[:, :w], AF.Copy)
            ssel = sb.tile([P, 8, 40], F32R, tag="ssel")
            nc.sync.dma_start(ssel[:], ssel_d[:, :, :])
            # hmT[d, j] = sum_t x_tm[t, d] * ssel[t, j]
            hmT = sb.tile([P, KD, 40], F32R, tag="hmT")
            for k in range(KD):
                pT = ps.tile([P, 512], F32, tag="ps")
                for tt in range(8):
                    nc.tensor.matmul(pT[:, :40],
                                     x_tm[:, tt, k * P:(k + 1) * P],
                                     ssel[:, tt], start=(tt == 0),
                                     stop=(tt == 7))
                nc.scalar.activation(hmT[:, k], pT[:, :40], AF.Copy)
            linb_sb = sb.tile([P, KD], F32, tag="linb")
            nc.sync.dma_start(linb_sb[:], lin_b[:, :])
            hml = sb.tile([P, KD, 40], F32, tag="hml")
            for mt in range(KD):
                lwc = sb2.tile([P, KD, P], F32R, tag="w1c")
                nc.sync.dma_start(lwc[:], lin_W[:, mt * P:(mt + 1) * P]
                                  .rearrange("(ko p) n -> p ko n", p=P))
                pp = ps.tile([P, 512], F32, tag="ps")
                for k in range(KD):
                    nc.tensor.matmul(pp[:, :40],
                                     lwc[:, k],
                                     hmT[:, k], start=(k == 0),
                                     stop=(k == KD - 1))
                nc.scalar.activation(hml[:, mt], pp[:, :40], AF.Gelu,
                                     bias=linb_sb[:, mt:mt + 1])
            nc.sync.dma_start(hm_out.rearrange("(k p) j -> p k j", p=P), hml[:])

    nc.compile()
    return nc


def _emit_layer(nc, sb, sb2, ps, l, xT,
                Wqkv, bqk, ln1_d, ln2_d, brow_d, Wo, W1, b1, W2,
                ident, ones512_d, ones512, onescol, epsc, mask_sb):
    # ---- per-layer small loads
    bqk_sb = sb2.tile([P, 12], F32, tag="bqk")
    nc.sync.dma_start(bqk_sb[:], bqk[l, :, :])
    ln1r = sb.tile([2, D], F32R, tag="ln1r")
    nc.sync.dma_start(ln1r[:], ln1_d[l, :, :])
    ln2r = sb.tile([2, D], F32R, tag="ln2r")
    nc.sync.dma_start(ln2r[:], ln2_d[l, :, :])
    brow = sb.tile([1, 3 * D], F32R, tag="brow")
    nc.sync.dma_start(brow[:], brow_d[l, :, :])
    b1_sb = sb2.tile([P, KF], F32, tag="b1")
    nc.sync.dma_start(b1_sb[:], b1[l, :, :])
    bvr = brow[0:1, 0:D]
    bor = brow[0:1, D:2 * D]
    b2r = brow[0:1, 2 * D:3 * D]

    # Wv: full [768, 768] resident (rhs for V); Wo full (lhsT)
    xhat1 = sb.tile([P, KD, T], F32R, tag="xhat1")

    for b in range(BL):
        # ---------- Q/K projections for batch b (feature-major)
        qkT = sb.tile([P, 12, S], F32R, tag="qkT")
        for mg in range(6):  # m-groups of 2 of the 12 q/k out-tiles
            wqk = sb2.tile([P, KD, 2 * P], F32R, tag="wqk")
            nc.sync.dma_start(
                wqk[:], Wqkv[l, :, mg * 2 * P:(mg + 1) * 2 * P]
                .rearrange("(ko p) n -> p ko n", p=P))
            for mj in range(2):
                m = mg * 2 + mj  # 0-5 = q tiles, 6-11 = k tiles
                pqk = ps.tile([P, 512], F32, tag="ps")
                for k in range(KD):
                    nc.tensor.matmul(pqk[:], wqk[:, k, mj * P:(mj + 1) * P],
                                     xT[:, k, b * S:(b + 1) * S],
                                     start=(k == 0), stop=(k == KD - 1))
                nc.scalar.activation(qkT[:, m], pqk[:], AF.Identity,
                                     bias=bqk_sb[:, m:m + 1])

        # ---------- V projection for batch b (token-major)
        v_tm = sb.tile([P, 4, D], F32R, tag="v_tm")
        for ng in range(2):
            w = 512 if ng == 0 else 256
            wvc = sb.tile([P, KD, 512], F32R, tag="wvc")
            nc.sync.dma_start(wvc[:, :, :w],
                              Wqkv[l, :, 2 * D + ng * 512:2 * D + ng * 512 + w]
                              .rearrange("(ko p) n -> p ko n", p=P))
            for st in range(4):
                pv = ps.tile([P, 512], F32, tag="ps")
                for k in range(KD):
                    nc.tensor.matmul(pv[:, :w],
                                     xT[:, k, b * S + st * P:b * S + (st + 1) * P],
                                     wvc[:, k, :w],
                                     start=(k == 0), stop=False)
                nc.tensor.matmul(pv[:, :w], ones512[:, :P],
                                 bvr[:, ng * 512:ng * 512 + w],
                                 start=False, stop=True)
                nc.scalar.activation(v_tm[:, st, ng * 512:ng * 512 + w],
                                     pv[:, :w], AF.Copy)

        # ---------- attention heads
        ctxT = sb.tile([P, KD, S], F32R, tag="ctxT")
        for h in range(H):
            jq = h // 2
            pb = (h % 2) * DK
            probs = sb.tile([P, 4, S], F32R, tag="probs")
            for mt in range(4):
                psc = ps.tile([P, 512], F32, tag="ps")
                nc.tensor.matmul(psc[:],
                                 qkT[pb:pb + DK, jq, mt * P:(mt + 1) * P],
                                 qkT[pb:pb + DK, 6 + jq, :],
                                 start=True, stop=True)
                nc.scalar.activation(probs[:, mt], psc[:], AF.Exp, scale=0.125)
            probsT = sb.tile([P, 4, S], F32R, tag="probsT")
            for kt in range(4):
                pT = ps.tile([P, 512], F32R, tag="ps")
                for mt in range(4):
                    nc.tensor.transpose(pT[:, mt * P:(mt + 1) * P],
                                        probs[:, mt, kt * P:(kt + 1) * P],
                                        ident[:])
                # masked copy out of PSUM (pad keys -> 0), split DVE/ACT
                if kt % 2 == 0:
                    nc.vector.tensor_tensor(probsT[:, kt], pT[:],
                                            mask_sb[:, b, kt:kt + 1]
                                            .to_broadcast([P, S]), ALU.mult)
                else:
                    nc.scalar.activation(probsT[:, kt], pT[:], AF.Copy,
                                         scale=mask_sb[:, b, kt:kt + 1])
            # denominators: column sums of probsT via PE, then reciprocal
            psum_r = ps.tile([P, 512], F32, tag="ps")
            for kt in range(4):
                nc.tensor.matmul(psum_r[:1], onescol[:], probsT[:, kt],
                                 start=(kt == 0), stop=(kt == 3))
            recip = sb2.tile([1, S], F32, tag="recip")
            nc.vector.reciprocal(recip[:], psum_r[:1])
            recip_r = sb2.tile([1, S], F32R, tag="recipr")
            nc.vector.tensor_copy(recip_r[:], recip[:])
            prb = ps.tile([P, 512], F32, tag="ps")
            nc.tensor.matmul(prb[:DK], ones512[:, :DK], recip_r[:],
                             start=True, stop=True)
            rb_sb = sb2.tile([DK, S], F32, tag="lnt1")
            nc.scalar.activation(rb_sb[:], prb[:DK], AF.Copy)
            # ctx.T[dv, tq] = sum_tk v[tk, dv] * probsT[tk, tq], then normalize
            pctx = ps.tile([P, 512], F32, tag="ps")
            for kt in range(4):
                nc.tensor.matmul(pctx[:DK], v_tm[:, kt, h * DK:(h + 1) * DK],
                                 probsT[:, kt], start=(kt == 0), stop=(kt == 3))
            nc.vector.tensor_tensor(ctxT[pb:pb + DK, jq], pctx[:DK], rb_sb[:],
                                    ALU.mult)

        # ---------- attention out projection + residual + LN1 for batch b
        h1 = sb.tile([P, KD, S], F32R, tag="v_tm")
        for m in range(KD):
            woch = sb2.tile([P, KD, P], F32R, tag="w1c")
            nc.sync.dma_start(woch[:], Wo[l, :, m * P:(m + 1) * P]
                              .rearrange("(ko p) n -> p ko n", p=P))
            po = ps.tile([P, 512], F32, tag="ps")
            for k in range(KD):
                nc.tensor.matmul(po[:], woch[:, k],
                                 ctxT[:, k], start=(k == 0), stop=False)
            nc.tensor.matmul(po[:], bor[:, m * P:(m + 1) * P], ones512[:],
                             start=False, stop=True)
            nc.vector.tensor_tensor(h1[:, m], po[:],
                                    xT[:, m, b * S:(b + 1) * S].bitcast(F32),
                                    ALU.add)
        _emit_ln(nc, sb, sb2, ps, h1, xhat1[:, :, b * S:(b + 1) * S], ln1r,
                 ones512_d, onescol, epsc)

    # ---------- feed-forward (fused FF1->FF2 per 512-token half) + LN2
    for b in range(BL):
        pf2 = [ps.tile([P, 512], F32, tag="ps", name=f"pf2_{m}")
               for m in range(KD)]
        for kk in range(KF):
            w1c = sb2.tile([P, KD, P], F32R, tag="w1c")
            nc.sync.dma_start(w1c[:], W1[l, :, kk * P:(kk + 1) * P]
                              .rearrange("(ko p) n -> p ko n", p=P))
            w2c = sb2.tile([P, D], F32R, tag="w2c")
            nc.sync.dma_start(w2c[:], W2[l, kk * P:(kk + 1) * P, :])
            pf1 = ps.tile([P, 512], F32, tag="ps")
            for k in range(KD):
                nc.tensor.matmul(pf1[:], w1c[:, k],
                                 xhat1[:, k, b * S:(b + 1) * S],
                                 start=(k == 0), stop=(k == KD - 1))
            f1 = sb2.tile([P, 512], F32R, tag="f1")
            nc.scalar.activation(f1[:], pf1[:], AF.Gelu,
                                 bias=b1_sb[:, kk:kk + 1])
            for m in range(KD):
                nc.tensor.matmul(pf2[m][:], w2c[:, m * P:(m + 1) * P], f1[:],
                                 start=(kk == 0), stop=False)
        h2 = sb.tile([P, KD, S], F32R, tag="v_tm")
        for m in range(KD):
            nc.tensor.matmul(pf2[m][:], b2r[:, m * P:(m + 1) * P], ones512[:],
                             start=False, stop=True)
            nc.vector.tensor_tensor(h2[:, m], pf2[m][:],
                                    xhat1[:, m, b * S:(b + 1) * S].bitcast(F32),
                                    ALU.add)
        _emit_ln(nc, sb, sb2, ps, h2, xT[:, :, b * S:(b + 1) * S], ln2r,
                 ones512_d, onescol, epsc)


def _emit_ln(nc, sb, sb2, ps, hin, xout, gbT, ones512_d, onescol, epsc):
    """LayerNorm over features (partition dim across KD tiles) of hin
    [P, KD, S] (F32R), writing g*(h-m)/sd + b into xout [P, KD, S]."""
    psx = ps.tile([P, 512], F32, tag="ps")
    for k in range(KD):
        nc.tensor.matmul(psx[:1], onescol[:], hin[:, k],
                         start=(k == 0), stop=(k == KD - 1))
    psx2 = ps.tile([P, 512], F32, tag="ps")
    for k in range(KD):
        sq = sb2.tile([P, 512], F32R, tag="f1")
        nc.scalar.activation(sq[:], hin[:, k], AF.Square)
        nc.tensor.matmul(psx2[:1], onescol[:], sq[:],
                         start=(k == 0), stop=(k == KD - 1))
    rowA = sb2.tile([1, S], F32, tag="lnA")   # m
    nc.vector.tensor_scalar(rowA[:], psx[:1], 1.0 / D, None, ALU.mult)
    rowB = sb2.tile([1, S], F32, tag="lnB")   # ex2 -> var -> rstd
    nc.vector.tensor_scalar(rowB[:], psx2[:1], 1.0 / D, None, ALU.mult)
    rowC = sb2.tile([1, S], F32, tag="lnC")   # m*m -> lnv -> mr
    nc.vector.tensor_tensor(rowC[:], rowA[:], rowA[:], ALU.mult)
    nc.vector.tensor_tensor(rowB[:], rowB[:], rowC[:], ALU.subtract)
    nc.scalar.activation(rowC[:], rowB[:], AF.Ln, bias=epsc[0:1, 0:1])
    rowR = sb.tile([1, S], F32R, tag="lnR")  # rstd; sole writer is ACT (f32r)
    nc.scalar.activation(rowR[:], rowC[:], AF.Exp, scale=-0.5)
    nc.vector.tensor_tensor(rowC[:], rowA[:], rowR[:].bitcast(F32),
                            ALU.mult)  # m*rstd
    rstd_r = rowR[:]
    # rows2 = [c ; ones] for the K=2 C' broadcast matmul
    rows2 = sb2.tile([2, S], F32R, tag="rows2")
    nc.scalar.activation(rows2[0:1], rowC[:], AF.Copy, scale=-1.0)
    nc.sync.dma_start(rows2[1:2], ones512_d[:, :])
    for k in range(KD):
        pA = ps.tile([P, 512], F32, tag="ps")
        nc.tensor.matmul(pA[:], gbT[0:1, k * P:(k + 1) * P], rstd_r,
                         start=True, stop=True)
        pC = ps.tile([P, 512], F32, tag="ps")
        nc.tensor.matmul(pC[:], gbT[:, k * P:(k + 1) * P], rows2[:],
                         start=True, stop=True)
        t1 = sb2.tile([P, 512], F32, tag="lnt1")
        nc.vector.tensor_tensor(t1[:], hin[:, k].bitcast(F32), pA[:], ALU.mult)
        nc.vector.tensor_tensor(xout[:, k], t1[:], pC[:], ALU.add)


# ---------------------------------------------------------------------------
# vocab-projection program (launch B): logits = h_all @ tokT_shard
# ---------------------------------------------------------------------------

def build_vocab():
    nc = bacc.Bacc("TRN2", target_bir_lowering=False, debug=False,
                   num_devices=NCORES)
    hallT = nc.dram_tensor("hallT", (D, 320), F32R, kind="ExternalInput")
    tokT = nc.dram_tensor("tokT", (D, VSHP), F32R, kind="ExternalInput")
    lm = nc.dram_tensor("lm", (320, VSHP), F32, kind="ExternalOutput")
    with tile.TileContext(nc) as tc:
        with tc.tile_pool(name="sb", bufs=1) as sb, \
             tc.tile_pool(name="sb2", bufs=3) as sb2, \
             tc.tile_pool(name="ps", bufs=8, space="PSUM") as ps:
            hall = sb.tile([P, KD, 320], F32R, tag="hall")
            nc.sync.dma_start(hall[:],
                              hallT.rearrange("(ko p) t -> p ko t", p=P))
            for nv in range(VSHP // 512):
                tc_sb = sb2.tile([P, KD, 512], F32R, tag="tokc")
                nc.sync.dma_start(tc_sb[:], tokT[:, nv * 512:(nv + 1) * 512]
                                  .rearrange("(ko p) n -> p ko n", p=P))
                for mt in range(3):
                    mw = 128 if mt < 2 else 64
                    pp = ps.tile([P, 512], F32, tag="ps")
                    for k in range(KD):
                        nc.tensor.matmul(pp[:mw],
                                         hall[:, k, mt * P:mt * P + mw],
                                         tc_sb[:, k], start=(k == 0),
                                         stop=(k == KD - 1))
                    ot = sb2.tile([P, 512], F32, tag="ot")
                    nc.scalar.activation(ot[:mw], pp[:mw], AF.Copy)
                    nc.sync.dma_start(lm[mt * P:mt * P + mw,
                                         nv * 512:(nv + 1) * 512], ot[:mw])
    nc.compile()
    return nc


# ---------------------------------------------------------------------------
# cached PJRT runner (compile once, reuse executable + device inputs)
# ---------------------------------------------------------------------------

class Runner:
    def __init__(self, nc, n_cores=NCORES):
        import jax
        from jax.sharding import Mesh, PartitionSpec
        from jax.experimental.shard_map import shard_map
        bass2jax.install_neuronx_cc_hook()
        self.nc = nc
        self.n_cores = n_cores
        partition_name = (nc.partition_id_tensor.name
                          if nc.partition_id_tensor else None)
        in_names, out_names, out_avals, zero_shapes = [], [], [], []
        for alloc in nc.m.functions[0].allocations:
            if not isinstance(alloc, mybir.MemoryLocationSet):
                continue
            name = alloc.memorylocations[0].name
            if alloc.kind == "ExternalInput":
                if name != partition_name:
                    in_names.append(name)
            elif alloc.kind == "ExternalOutput":
                shape = tuple(alloc.tensor_shape)
                dtype = mybir.dt.np(alloc.dtype)
                out_names.append(name)
                out_avals.append(jax.core.ShapedArray(shape, dtype))
                zero_shapes.append((shape, dtype))
        self.n_params = len(in_names)
        self.in_names = list(in_names)
        self.out_names = out_names
        self.out_avals = out_avals
        self.zero_shapes = zero_shapes
        all_in = in_names + out_names
        if partition_name is not None:
            all_in = all_in + [partition_name]

        def _body(*args):
            operands = list(args)
            if partition_name is not None:
                operands.append(bass2jax.partition_id_tensor())
            outs = bass2jax._bass_exec_p.bind(
                *operands,
                out_avals=tuple(out_avals),
                in_names=tuple(all_in),
                out_names=tuple(out_names),
                lowering_input_output_aliases=(),
                sim_require_finite=True,
                sim_require_nnan=True,
                nc=nc,
            )
            return tuple(outs)

        devices = jax.devices()[:n_cores]
        self.mesh = Mesh(np.asarray(devices), ("core",))
        n_outs = len(out_names)
        donate = tuple(range(self.n_params, self.n_params + n_outs))
        self.fn = jax.jit(
            shard_map(_body, mesh=self.mesh,
                      in_specs=(PartitionSpec("core"),) * (self.n_params + n_outs),
                      out_specs=(PartitionSpec("core"),) * n_outs,
                      check_rep=False),
            donate_argnums=donate, keep_unused=True)

    def put_inputs(self, in_maps):
        import jax
        from jax.sharding import NamedSharding, PartitionSpec
        sh = NamedSharding(self.mesh, PartitionSpec("core"))
        out = []
        for name in self.in_names:
            a = np.concatenate([np.asarray(m[name]) for m in in_maps], axis=0)
            out.append(jax.device_put(a, sh))
        return out

    def zeros(self):
        import jax.numpy as jnp
        from jax.sharding import NamedSharding, PartitionSpec
        sh = NamedSharding(self.mesh, PartitionSpec("core"))
        return [jnp.zeros((self.n_cores * s[0], *s[1:]), d, device=sh)
                for (s, d) in self.zero_shapes]

    def run(self, dev_in):
        import jax
        outs = self.fn(*dev_in, *self.zeros())
        jax.block_until_ready(outs)
        return outs

    def split(self, out_arrs):
        res = []
        for c in range(self.n_cores):
            res.append({name: np.asarray(out_arrs[i])
                        .reshape(self.n_cores, *self.out_avals[i].shape)[c]
                        for i, name in enumerate(self.out_names)})
        return res

    def __call__(self, in_maps):
        return self.split(self.run(self.put_inputs(in_maps)))


# ---------------------------------------------------------------------------
# host-side preparation + execution
# ---------------------------------------------------------------------------

_CACHE = {}


def _prep_shared(params, nl=NL):
    """Build the shared (non-per-core) input arrays from params."""
    Lp = params['layers']
    g = {}
    tok = np.ascontiguousarray(np.asarray(params['tok'], dtype=np.float32))
    g['tok'] = tok
    g['pos'] = np.ascontiguousarray(np.asarray(params['pos'], np.float32)[:S])
    g['seg'] = np.ascontiguousarray(np.asarray(params['seg'], np.float32))
    eg = np.asarray(params['emb_g'], np.float32)
    eb = np.asarray(params['emb_b'], np.float32)
    g['emb_gb'] = np.ascontiguousarray(
        np.stack([eg.reshape(KD, P).T, eb.reshape(KD, P).T], axis=2))
    g['ident'] = round_fp32r(np.eye(P, dtype=np.float32))
    g['ones512'] = round_fp32r(np.ones((1, 512), np.float32))
    g['onescol'] = round_fp32r(np.ones((P, 1), np.float32))
    g['epsc'] = np.full((P, 1), 1e-5, np.float32)

    Wq = np.asarray(Lp['Wq'], np.float32)[:nl]
    Wk = np.asarray(Lp['Wk'], np.float32)[:nl]
    Wv = np.asarray(Lp['Wv'], np.float32)[:nl]
    g['Wqkv'] = round_fp32r(np.concatenate([Wq, Wk, Wv], axis=2))
    bq = np.asarray(Lp['bq'], np.float32)[:nl].reshape(nl, KD, P)
    bk = np.asarray(Lp['bk'], np.float32)[:nl].reshape(nl, KD, P)
    g['bqk'] = np.ascontiguousarray(
        np.concatenate([bq, bk], axis=1).transpose(0, 2, 1))
    g['ln1_gb'] = round_fp32r(np.stack(
        [np.asarray(Lp['ln1_g'], np.float32)[:nl],
         np.asarray(Lp['ln1_b'], np.float32)[:nl]], axis=1))
    g['ln2_gb'] = round_fp32r(np.stack(
        [np.asarray(Lp['ln2_g'], np.float32)[:nl],
         np.asarray(Lp['ln2_b'], np.float32)[:nl]], axis=1))
    g['brow'] = round_fp32r(np.concatenate(
        [np.asarray(Lp['bv'], np.float32)[:nl],
         np.asarray(Lp['bo'], np.float32)[:nl],
         np.asarray(Lp['b2'], np.float32)[:nl]], axis=1)[:, None, :])
    g['Wo'] = round_fp32r(np.asarray(Lp['Wo'], np.float32)[:nl])
    g['W1'] = round_fp32r(np.asarray(Lp['W1'], np.float32)[:nl])
    g['b1'] = np.ascontiguousarray(
        np.asarray(Lp['b1'], np.float32)[:nl].reshape(nl, KF, P)
        .transpose(0, 2, 1))
    g['W2'] = round_fp32r(np.asarray(Lp['W2'], np.float32)[:nl])
    g['pool_W'] = round_fp32r(np.asarray(params['pool_W'], np.float32))
    g['pool_b'] = np.ascontiguousarray(
        np.asarray(params['pool_b'], np.float32).reshape(KD, P).T)
    g['cls_W'] = round_fp32r(np.asarray(params['cls_W'], np.float32))
    g['cls_b'] = np.asarray(params['cls_b'], np.float32).reshape(2, 1)
    g['lin_W'] = round_fp32r(np.asarray(params['lin_W'], np.float32))
    g['lin_b'] = np.ascontiguousarray(
        np.asarray(params['lin_b'], np.float32).reshape(KD, P).T)
    return g


def make_in_maps(ids, segs, mp, g):
    in_maps = []
    for c in range(NCORES):
        im = dict(g)
        cid = ids[c * BL:(c + 1) * BL]
        cseg = segs[c * BL:(c + 1) * BL]
        cmp = mp[c * BL:(c + 1) * BL]
        im['ids_idx'] = np.ascontiguousarray(cid.reshape(8, P).T)
        im['seg_idx'] = np.ascontiguousarray(cseg.reshape(8, P).T)
        mpg = np.concatenate([cmp[0], cmp[1] + S]).astype(np.int64)
        ssel = np.zeros((P, 8, 40), np.float32)
        for j, t in enumerate(mpg):
            ssel[t % P, t // P, j] = 1.0
        im['ssel'] = ssel
        m01 = (cid != 0).astype(np.float32)
        im['mask01'] = np.ascontiguousarray(
            m01.reshape(BL, 4, P).transpose(2, 0, 1))
        in_maps.append(im)
    return in_maps


def kernel(input_ids, segment_ids, masked_pos, params):
    ids = np.asarray(input_ids).astype(np.int32)
    segs = np.asarray(segment_ids).astype(np.int32)
    mp = np.asarray(masked_pos).astype(np.int32)
    B = ids.shape[0]
    assert B == NCORES * BL

    if 'main_r' not in _CACHE:
        _CACHE['main_r'] = Runner(build_main())
    if 'vocab_r' not in _CACHE:
        _CACHE['vocab_r'] = Runner(build_vocab())

    g = _prep_shared(params)
    in_maps = make_in_maps(ids, segs, mp, g)

    results = _CACHE['main_r'](in_maps)
    hm = np.stack([results[c]['hm_out'] for c in range(NCORES)])
    clsf = np.concatenate([results[c]['clsf_out'].T
                           for c in range(NCORES)])

    # vocab projection launch (vocab-sharded over the 8 cores)
    hallT = round_fp32r(np.concatenate(list(hm), axis=1))  # [768, 320]
    in_maps2 = []
    for c in range(NCORES):
        tksh = np.zeros((D, VSHP), np.float32)
        tksh[:, :VSH] = g['tok'][c * VSH:(c + 1) * VSH, :].T
        in_maps2.append({'hallT': hallT, 'tokT': round_fp32r(tksh)})
    results2 = _CACHE['vocab_r'](in_maps2)
    lm = np.concatenate([results2[c]['lm'][:, :VSH]
                         for c in range(NCORES)], axis=1)
    logits_lm = lm.reshape(B, NPRED, V)
    return logits_lm, clsf


# revision 22
# speedup vs baseline: 121.4077x; 5.0844x over previous
"""BERT-base forward on 8 Trainium2 NeuronCores (Bass/Tile).

Strategy: data-parallel over batch (B=16 -> 2 per core) for the embedding +
12 transformer layers + pooler/cls + MLM gather/transform; the tied vocab
projection is sharded over the vocab axis in a second small SPMD launch
(each core: all 320 masked positions x 4000 vocab rows).

Layouts (per core, T = 2*512 = 1024 local tokens):
  - activations are feature-major: xT[p, k, t] = x[t, k*128+p]  (d = 6 tiles)
  - weights stay in natural [d_in, d_out] layout -> they are the PE lhsT
  - V is produced token-major for the attn@V matmul's lhsT
  - matmuls run in float32r (fp32 rounded to 11-bit mantissa, full PE speed)
"""
import sys
if '/opt/trn_rl_repo' not in sys.path:
    sys.path.insert(0, '/opt/trn_rl_repo')

import contextlib

import numpy as np

import concourse.bass as bass
import concourse.mybir as mybir
import concourse.tile as tile
from concourse import bacc, bass_utils, bass2jax

P = 128
D = 768
KD = 6            # d tiles
H = 12
DK = 64
DFF = 3072
KF = 24           # dff tiles
NL = 12
S = 512
BL = 2            # local batch
T = BL * S        # local tokens
NPRED = 20
V = 32000
VSH = 4000        # vocab shard per core
VSHP = 4096       # padded
NCORES = 8

F32 = mybir.dt.float32
F32R = mybir.dt.float32r
I32 = mybir.dt.int32
AF = mybir.ActivationFunctionType
ALU = mybir.AluOpType
AX = mybir.AxisListType


def round_fp32r(x: np.ndarray) -> np.ndarray:
    """Round fp32 to the fp32r grid (11-bit mantissa, RN-even)."""
    u = np.ascontiguousarray(x, dtype=np.float32).view(np.uint32)
    r = (u.astype(np.uint64) + 0x7FF + ((u >> 12) & 1)) & 0xFFFFF000
    return r.astype(np.uint32).view(np.float32)


# ---------------------------------------------------------------------------
# main program (embedding + 12 layers + pooler/cls + MLM transform)
# ---------------------------------------------------------------------------

def build_main(nl=NL, debug=False):
    nc = bacc.Bacc("TRN2", target_bir_lowering=False, debug=False,
                   num_devices=NCORES)

    # ---- inputs
    tok = nc.dram_tensor("tok", (V, D), F32, kind="ExternalInput")
    pos = nc.dram_tensor("pos", (S, D), F32, kind="ExternalInput")
    seg = nc.dram_tensor("seg", (2, D), F32, kind="ExternalInput")
    ids_idx = nc.dram_tensor("ids_idx", (P, 8), I32, kind="ExternalInput")
    seg_idx = nc.dram_tensor("seg_idx", (P, 8), I32, kind="ExternalInput")
    ssel_d = nc.dram_tensor("ssel", (P, 8, 40), F32R, kind="ExternalInput")
    emb_gb = nc.dram_tensor("emb_gb", (P, KD, 2), F32, kind="ExternalInput")
    mask01 = nc.dram_tensor("mask01", (P, BL, 4), F32, kind="ExternalInput")
    ident_d = nc.dram_tensor("ident", (P, P), F32R, kind="ExternalInput")
    ones512_d = nc.dram_tensor("ones512", (1, 512), F32R, kind="ExternalInput")
    onescol_d = nc.dram_tensor("onescol", (P, 1), F32R, kind="ExternalInput")
    eps_d = nc.dram_tensor("epsc", (P, 1), F32, kind="ExternalInput")

    Wqkv = nc.dram_tensor("Wqkv", (nl, D, 3 * D), F32R, kind="ExternalInput")
    bqk = nc.dram_tensor("bqk", (nl, P, 12), F32, kind="ExternalInput")
    ln1_d = nc.dram_tensor("ln1_gb", (nl, 2, D), F32R, kind="ExternalInput")
    ln2_d = nc.dram_tensor("ln2_gb", (nl, 2, D), F32R, kind="ExternalInput")
    # bias rows free-packed: [0:768]=bv, [768:1536]=bo, [1536:2304]=b2
    brow_d = nc.dram_tensor("brow", (nl, 1, 3 * D), F32R, kind="ExternalInput")
    Wo = nc.dram_tensor("Wo", (nl, D, D), F32R, kind="ExternalInput")
    W1 = nc.dram_tensor("W1", (nl, D, DFF), F32R, kind="ExternalInput")
    b1 = nc.dram_tensor("b1", (nl, P, KF), F32, kind="ExternalInput")
    W2 = nc.dram_tensor("W2", (nl, DFF, D), F32R, kind="ExternalInput")

    pool_W = nc.dram_tensor("pool_W", (D, D), F32R, kind="ExternalInput")
    pool_b = nc.dram_tensor("pool_b", (P, KD), F32, kind="ExternalInput")
    cls_W = nc.dram_tensor("cls_W", (D, 2), F32R, kind="ExternalInput")
    cls_b = nc.dram_tensor("cls_b", (2, 1), F32, kind="ExternalInput")
    lin_W = nc.dram_tensor("lin_W", (D, D), F32R, kind="ExternalInput")
    lin_b = nc.dram_tensor("lin_b", (P, KD), F32, kind="ExternalInput")

    # ---- outputs
    clsf_out = nc.dram_tensor("clsf_out", (2, BL), F32, kind="ExternalOutput")
    hm_out = nc.dram_tensor("hm_out", (D, 40), F32, kind="ExternalOutput")
    dbg = {}
    if debug:
        dbg['emb'] = nc.dram_tensor("dbg_emb", (D, T), F32, kind="ExternalOutput")
        dbg['hm_tm'] = nc.dram_tensor("dbg_hmtm", (P, D), F32, kind="ExternalOutput")
        dbg['x_tm'] = nc.dram_tensor("dbg_xtm", (T, D), F32, kind="ExternalOutput")
        for l in range(nl):
            dbg[f'x{l}'] = nc.dram_tensor(f"dbg_x{l}", (D, T), F32,
                                          kind="ExternalOutput")

    with tile.TileContext(nc) as tc:
        with contextlib.ExitStack() as ctx:
            sb = ctx.enter_context(tc.tile_pool(name="sb", bufs=1))
            sb2 = ctx.enter_context(tc.tile_pool(name="sb2", bufs=2))
            ps = ctx.enter_context(tc.tile_pool(name="ps", bufs=8, space="PSUM"))
            dramp = ctx.enter_context(tc.tile_pool(name="dram", bufs=1,
                                                   space="DRAM"))

            # ---------- persistent constants
            ident = sb.tile([P, P], F32R, tag="ident")
            nc.sync.dma_start(ident[:], ident_d[:, :])
            ones512 = sb.tile([1, 512], F32R, tag="ones512")
            nc.sync.dma_start(ones512[:], ones512_d[:, :])
            onescol = sb.tile([P, 1], F32R, tag="onescol")
            nc.sync.dma_start(onescol[:], onescol_d[:, :])
            epsc = sb.tile([P, 1], F32, tag="epsc")
            nc.sync.dma_start(epsc[:], eps_d[:, :])
            mask_sb = sb.tile([P, BL, 4], F32, tag="mask")
            nc.sync.dma_start(mask_sb[:], mask01[:, :, :])
            embgb = sb.tile([P, KD, 2], F32, tag="embgb")
            nc.sync.dma_start(embgb[:], emb_gb[:, :, :])

            # persistent activation buffer (updated in place across layers)
            xT = sb.tile([P, KD, T], F32R, tag="xT")

            # ---------- embedding (token-major), then transpose into xT
            idx_sb = sb.tile([P, 8], I32, tag="idx")
            nc.sync.dma_start(idx_sb[:], ids_idx[:, :])
            sidx_sb = sb.tile([P, 8], I32, tag="sidx")
            nc.sync.dma_start(sidx_sb[:], seg_idx[:, :])

            for tt in range(8):
                x0 = sb2.tile([P, D], F32, tag="wqk")
                nc.gpsimd.indirect_dma_start(
                    out=x0[:], out_offset=None, in_=tok[:],
                    in_offset=bass.IndirectOffsetOnAxis(ap=idx_sb[:, tt:tt + 1],
                                                        axis=0))
                nc.gpsimd.indirect_dma_start(
                    out=x0[:], out_offset=None, in_=seg[:],
                    in_offset=bass.IndirectOffsetOnAxis(ap=sidx_sb[:, tt:tt + 1],
                                                        axis=0),
                    compute_op=ALU.add)
                so = (tt % 4) * P
                nc.gpsimd.dma_start(x0[:], pos[so:so + P, :], accum_op=ALU.add)
                # LayerNorm over free dim (emb g/b applied after the transpose)
                sx = sb2.tile([P, 1], F32, tag="row_s")
                nc.vector.reduce_sum(sx[:], x0[:], axis=AX.X)
                sq = sb2.tile([P, D], F32, tag="w2c")
                nc.vector.tensor_tensor(sq[:], x0[:], x0[:], ALU.mult)
                sx2 = sb2.tile([P, 1], F32, tag="row_s2")
                nc.vector.reduce_sum(sx2[:], sq[:], axis=AX.X)
                m = sb2.tile([P, 1], F32, tag="row_m")
                nc.vector.tensor_scalar(m[:], sx[:], 1.0 / D, None, ALU.mult)
                ex2 = sb2.tile([P, 1], F32, tag="row_e")
                nc.vector.tensor_scalar(ex2[:], sx2[:], 1.0 / D, None, ALU.mult)
                var = sb2.tile([P, 1], F32, tag="row_v")
                nc.vector.tensor_tensor(var[:], m[:], m[:], ALU.mult)
                nc.vector.tensor_tensor(var[:], ex2[:], var[:], ALU.subtract)
                lnv = sb2.tile([P, 1], F32, tag="row_l")
                nc.scalar.activation(lnv[:], var[:], AF.Ln, bias=epsc[:, 0:1])
                rstd = sb2.tile([P, 1], F32, tag="row_r")
                nc.scalar.activation(rstd[:], lnv[:], AF.Exp, scale=-0.5)
                xc = sb2.tile([P, D], F32, tag="w2c")
                nc.vector.tensor_tensor(xc[:], x0[:],
                                        m[:].to_broadcast([P, D]), ALU.subtract)
                xh = sb2.tile([P, D], F32R, tag="w1c")
                nc.vector.tensor_tensor(xh[:], xc[:],
                                        rstd[:].to_broadcast([P, D]), ALU.mult)
                # transpose this token tile into feature-major, fusing emb g/b
                for kg in range(2):
                    nk = 4 if kg == 0 else 2
                    pT = ps.tile([P, 512], F32R, tag="ps")
                    for j in range(nk):
                        k = kg * 4 + j
                        nc.tensor.transpose(pT[:, j * P:(j + 1) * P],
                                            xh[:, k * P:(k + 1) * P], ident[:])
                    for j in range(nk):
                        k = kg * 4 + j
                        nc.scalar.activation(
                            xT[:, k, tt * P:(tt + 1) * P],
                            pT[:, j * P:(j + 1) * P], AF.Identity,
                            scale=embgb[:, k, 0:1], bias=embgb[:, k, 1:2])

            if debug:
                nc.sync.dma_start(dbg['emb'].rearrange("(k p) t -> p k t", p=P),
                                  xT[:].bitcast(F32))

            # ---------- transformer layers
            for l in range(nl):
                _emit_layer(nc, sb, sb2, ps, l, xT,
                            Wqkv, bqk, ln1_d, ln2_d, brow_d, Wo, W1, b1, W2,
                            ident, ones512_d, ones512, onescol, epsc, mask_sb)
                if debug:
                    nc.sync.dma_start(
                        dbg[f'x{l}'].rearrange("(k p) t -> p k t", p=P),
                        xT[:].bitcast(F32))

            # ---------- pooler + classifier
            hpT = sb.tile([P, KD, BL], F32R, tag="hpT")
            poolb_sb = sb.tile([P, KD], F32, tag="poolb")
            nc.sync.dma_start(poolb_sb[:], pool_b[:, :])
            x0T = xT[:, :, 0:S + 1:S]  # tokens 0 and 512 (CLS of both batches)
            for mt in range(KD):
                pwc = sb2.tile([P, KD, P], F32R, tag="w1c")
                nc.sync.dma_start(pwc[:], pool_W[:, mt * P:(mt + 1) * P]
                                  .rearrange("(ko p) n -> p ko n", p=P))
                pp = ps.tile([P, 512], F32, tag="ps")
                for k in range(KD):
                    nc.tensor.matmul(pp[:, :BL],
                                     pwc[:, k],
                                     x0T[:, k], start=(k == 0),
                                     stop=(k == KD - 1))
                nc.scalar.activation(hpT[:, mt], pp[:, :BL], AF.Tanh,
                                     bias=poolb_sb[:, mt:mt + 1])
            clsW_sb = sb.tile([P, KD, 2], F32R, tag="clsW")
            nc.sync.dma_start(clsW_sb[:],
                              cls_W.rearrange("(ko p) n -> p ko n", p=P))
            clsb_sb = sb.tile([2, 1], F32, tag="clsb")
            nc.sync.dma_start(clsb_sb[:], cls_b[:, :])
            pc = ps.tile([P, 512], F32, tag="ps")
            for k in range(KD):
                nc.tensor.matmul(pc[:2, :BL], clsW_sb[:, k], hpT[:, k],
                                 start=(k == 0), stop=(k == KD - 1))
            clsf_sb = sb.tile([2, BL], F32, tag="clsf")
            nc.scalar.activation(clsf_sb[:], pc[:2, :BL], AF.Identity,
                                 bias=clsb_sb[:, 0:1])
            nc.sync.dma_start(clsf_out[:, :], clsf_sb[:])

            # ---------- MLM: transpose final x, select masked rows via PE
            x_tm = sb.tile([P, 8, D], F32R, tag="qkT")
            for tt in range(8):
                for kg in range(2):
                    w = 512 if kg == 0 else 256
                    nk = 4 if kg == 0 else 2
                    pT = ps.tile([P, 512], F32R, tag="ps")
                    for j in range(nk):
                        k = kg * 4 + j
                        nc.tensor.transpose(pT[:, j * P:(j + 1) * P],
                                            xT[:, k, tt * P:(tt + 1) * P],
                                            ident[:])
                    nc.scalar.activation(x_tm[:, tt, kg * 512:kg * 512 + w],
                                         pT[:, :w], AF.Copy)
            ssel = sb.tile([P, 8, 40], F32R, tag="ssel")
            nc.sync.dma_start(ssel[:], ssel_d[:, :, :])
            # hmT[d, j] = sum_t x_tm[t, d] * ssel[t, j]
            hmT = sb.tile([P, KD, 40], F32R, tag="hmT")
            for k in range(KD):
                pT = ps.tile([P, 512], F32, tag="ps")
                for tt in range(8):
                    nc.tensor.matmul(pT[:, :40],
                                     x_tm[:, tt, k * P:(k + 1) * P],
                                     ssel[:, tt], start=(tt == 0),
                                     stop=(tt == 7))
                nc.scalar.activation(hmT[:, k], pT[:, :40], AF.Copy)
            linb_sb = sb.tile([P, KD], F32, tag="linb")
            nc.sync.dma_start(linb_sb[:], lin_b[:, :])
            hml = sb.tile([P, KD, 40], F32, tag="hml")
            for mt in range(KD):
                lwc = sb2.tile([P, KD, P], F32R, tag="w1c")
                nc.sync.dma_start(lwc[:], lin_W[:, mt * P:(mt + 1) * P]
                                  .rearrange("(ko p) n -> p ko n", p=P))
                pp = ps.tile([P, 512], F32, tag="ps")
                for k in range(KD):
                    nc.tensor.matmul(pp[:, :40],
                                     lwc[:, k],
                                     hmT[:, k], start=(k == 0),
                                     stop=(k == KD - 1))
                nc.scalar.activation(hml[:, mt], pp[:, :40], AF.Gelu,
                                     bias=linb_sb[:, mt:mt + 1])
            nc.sync.dma_start(hm_out.rearrange("(k p) j -> p k j", p=P), hml[:])

    nc.compile()
    return nc


def _emit_layer(nc, sb, sb2, ps, l, xT,
                Wqkv, bqk, ln1_d, ln2_d, brow_d, Wo, W1, b1, W2,
                ident, ones512_d, ones512, onescol, epsc, mask_sb):
    # ---- per-layer small loads
    bqk_sb = sb2.tile([P, 12], F32, tag="bqk")
    nc.sync.dma_start(bqk_sb[:], bqk[l, :, :])
    ln1r = sb.tile([2, D], F32R, tag="ln1r")
    nc.sync.dma_start(ln1r[:], ln1_d[l, :, :])
    ln2r = sb.tile([2, D], F32R, tag="ln2r")
    nc.sync.dma_start(ln2r[:], ln2_d[l, :, :])
    brow = sb.tile([1, 3 * D], F32R, tag="brow")
    nc.sync.dma_start(brow[:], brow_d[l, :, :])
    b1_sb = sb2.tile([P, KF], F32, tag="b1")
    nc.sync.dma_start(b1_sb[:], b1[l, :, :])
    bvr = brow[0:1, 0:D]
    bor = brow[0:1, D:2 * D]
    b2r = brow[0:1, 2 * D:3 * D]

    # Wv: full [768, 768] resident (rhs for V); Wo full (lhsT)
    xhat1 = sb.tile([P, KD, T], F32R, tag="xhat1")

    for b in range(BL):
        # ---------- Q/K projections for batch b (feature-major)
        qkT = sb.tile([P, 12, S], F32R, tag="qkT")
        for mg in range(6):  # m-groups of 2 of the 12 q/k out-tiles
            wqk = sb2.tile([P, KD, 2 * P], F32R, tag="wqk")
            nc.sync.dma_start(
                wqk[:], Wqkv[l, :, mg * 2 * P:(mg + 1) * 2 * P]
                .rearrange("(ko p) n -> p ko n", p=P))
            for mj in range(2):
                m = mg * 2 + mj  # 0-5 = q tiles, 6-11 = k tiles
                pqk = ps.tile([P, 512], F32, tag="ps")
                for k in range(KD):
                    nc.tensor.matmul(pqk[:], wqk[:, k, mj * P:(mj + 1) * P],
                                     xT[:, k, b * S:(b + 1) * S],
                                     start=(k == 0), stop=(k == KD - 1))
                nc.scalar.activation(qkT[:, m], pqk[:], AF.Identity,
                                     bias=bqk_sb[:, m:m + 1])

        # ---------- V projection for batch b (token-major)
        v_tm = sb.tile([P, 4, D], F32R, tag="v_tm")
        for ng in range(2):
            w = 512 if ng == 0 else 256
            wvc = sb.tile([P, KD, 512], F32R, tag="wvc")
            nc.sync.dma_start(wvc[:, :, :w],
                              Wqkv[l, :, 2 * D + ng * 512:2 * D + ng * 512 + w]
                              .rearrange("(ko p) n -> p ko n", p=P))
            for st in range(4):
                pv = ps.tile([P, 512], F32, tag="ps")
                for k in range(KD):
                    nc.tensor.matmul(pv[:, :w],
                                     xT[:, k, b * S + st * P:b * S + (st + 1) * P],
                                     wvc[:, k, :w],
                                     start=(k == 0), stop=False)
                nc.tensor.matmul(pv[:, :w], ones512[:, :P],
                                 bvr[:, ng * 512:ng * 512 + w],
                                 start=False, stop=True)
                nc.scalar.activation(v_tm[:, st, ng * 512:ng * 512 + w],
                                     pv[:, :w], AF.Copy)

        # ---------- attention heads
        ctxT = sb.tile([P, KD, S], F32R, tag="ctxT")
        for h in range(H):
            jq = h // 2
            pb = (h % 2) * DK
            probs = sb.tile([P, 4, S], F32R, tag="probs")
            for mt in range(4):
                psc = ps.tile([P, 512], F32, tag="ps")
                nc.tensor.matmul(psc[:],
                                 qkT[pb:pb + DK, jq, mt * P:(mt + 1) * P],
                                 qkT[pb:pb + DK, 6 + jq, :],
                                 start=True, stop=True)
                nc.scalar.activation(probs[:, mt], psc[:], AF.Exp, scale=0.125)
            probsT = sb.tile([P, 4, S], F32R, tag="probsT")
            for kt in range(4):
                pT = ps.tile([P, 512], F32R, tag="ps")
                for mt in range(4):
                    nc.tensor.transpose(pT[:, mt * P:(mt + 1) * P],
                                        probs[:, mt, kt * P:(kt + 1) * P],
                                        ident[:])
                # masked copy out of PSUM (pad keys -> 0), split DVE/ACT
                if kt % 2 == 0:
                    nc.vector.tensor_tensor(probsT[:, kt], pT[:],
                                            mask_sb[:, b, kt:kt + 1]
                                            .to_broadcast([P, S]), ALU.mult)
                else:
                    nc.scalar.activation(probsT[:, kt], pT[:], AF.Copy,
                                         scale=mask_sb[:, b, kt:kt + 1])
            # denominators: column sums of probsT via PE, then reciprocal
            psum_r = ps.tile([P, 512], F32, tag="ps")
            for kt in range(4):
                nc.tensor.matmul(psum_r[:1], onescol[:], probsT[:, kt],
                                 start=(kt == 0), stop=(kt == 3))
            recip = sb2.tile([1, S], F32, tag="recip")
            nc.vector.reciprocal(recip[:], psum_r[:1])
            recip_r = sb2.tile([1, S], F32R, tag="recipr")
            nc.vector.tensor_copy(recip_r[:], recip[:])
            prb = ps.tile([P, 512], F32, tag="ps")
            nc.tensor.matmul(prb[:DK], ones512[:, :DK], recip_r[:],
                             start=True, stop=True)
            rb_sb = sb2.tile([DK, S], F32, tag="lnt1")
            nc.scalar.activation(rb_sb[:], prb[:DK], AF.Copy)
            # ctx.T[dv, tq] = sum_tk v[tk, dv] * probsT[tk, tq], then normalize
            pctx = ps.tile([P, 512], F32, tag="ps")
            for kt in range(4):
                nc.tensor.matmul(pctx[:DK], v_tm[:, kt, h * DK:(h + 1) * DK],
                                 probsT[:, kt], start=(kt == 0), stop=(kt == 3))
            nc.vector.tensor_tensor(ctxT[pb:pb + DK, jq], pctx[:DK], rb_sb[:],
                                    ALU.mult)

        # ---------- attention out projection + residual + LN1 for batch b
        h1 = sb.tile([P, KD, S], F32R, tag="v_tm")
        for m in range(KD):
            woch = sb2.tile([P, KD, P], F32R, tag="w1c")
            nc.sync.dma_start(woch[:], Wo[l, :, m * P:(m + 1) * P]
                              .rearrange("(ko p) n -> p ko n", p=P))
            po = ps.tile([P, 512], F32, tag="ps")
            for k in range(KD):
                nc.tensor.matmul(po[:], woch[:, k],
                                 ctxT[:, k], start=(k == 0), stop=False)
            nc.tensor.matmul(po[:], bor[:, m * P:(m + 1) * P], ones512[:],
                             start=False, stop=True)
            nc.vector.tensor_tensor(h1[:, m], po[:],
                                    xT[:, m, b * S:(b + 1) * S].bitcast(F32),
                                    ALU.add)
        _emit_ln(nc, sb, sb2, ps, h1, xhat1[:, :, b * S:(b + 1) * S], ln1r,
                 ones512_d, onescol, epsc)

    # ---------- feed-forward (fused FF1->FF2 per 512-token half) + LN2
    for b in range(BL):
        pf2 = [ps.tile([P, 512], F32, tag="ps", name=f"pf2_{m}")
               for m in range(KD)]
        for kk in range(KF):
            w1c = sb2.tile([P, KD, P], F32R, tag="w1c")
            nc.sync.dma_start(w1c[:], W1[l, :, kk * P:(kk + 1) * P]
                              .rearrange("(ko p) n -> p ko n", p=P))
            w2c = sb2.tile([P, D], F32R, tag="w2c")
            nc.sync.dma_start(w2c[:], W2[l, kk * P:(kk + 1) * P, :])
            pf1 = ps.tile([P, 512], F32, tag="ps")
            for k in range(KD):
                nc.tensor.matmul(pf1[:], w1c[:, k],
                                 xhat1[:, k, b * S:(b + 1) * S],
                                 start=(k == 0), stop=(k == KD - 1))
            f1 = sb2.tile([P, 512], F32R, tag="f1")
            nc.scalar.activation(f1[:], pf1[:], AF.Gelu,
                                 bias=b1_sb[:, kk:kk + 1])
            for m in range(KD):
                nc.tensor.matmul(pf2[m][:], w2c[:, m * P:(m + 1) * P], f1[:],
                                 start=(kk == 0), stop=False)
        h2 = sb.tile([P, KD, S], F32R, tag="v_tm")
        for m in range(KD):
            nc.tensor.matmul(pf2[m][:], b2r[:, m * P:(m + 1) * P], ones512[:],
                             start=False, stop=True)
            nc.vector.tensor_tensor(h2[:, m], pf2[m][:],
                                    xhat1[:, m, b * S:(b + 1) * S].bitcast(F32),
                                    ALU.add)
        _emit_ln(nc, sb, sb2, ps, h2, xT[:, :, b * S:(b + 1) * S], ln2r,
                 ones512_d, onescol, epsc)


def _emit_ln(nc, sb, sb2, ps, hin, xout, gbT, ones512_d, onescol, epsc):
    """LayerNorm over features (partition dim across KD tiles) of hin
    [P, KD, S] (F32R), writing g*(h-m)/sd + b into xout [P, KD, S]."""
    psx = ps.tile([P, 512], F32, tag="ps")
    for k in range(KD):
        nc.tensor.matmul(psx[:1], onescol[:], hin[:, k],
                         start=(k == 0), stop=(k == KD - 1))
    psx2 = ps.tile([P, 512], F32, tag="ps")
    for k in range(KD):
        sq = sb2.tile([P, 512], F32R, tag="f1")
        nc.scalar.activation(sq[:], hin[:, k], AF.Square)
        nc.tensor.matmul(psx2[:1], onescol[:], sq[:],
                         start=(k == 0), stop=(k == KD - 1))
    rowA = sb2.tile([1, S], F32, tag="lnA")   # m
    nc.vector.tensor_scalar(rowA[:], psx[:1], 1.0 / D, None, ALU.mult)
    rowB = sb2.tile([1, S], F32, tag="lnB")   # ex2 -> var -> rstd
    nc.vector.tensor_scalar(rowB[:], psx2[:1], 1.0 / D, None, ALU.mult)
    rowC = sb2.tile([1, S], F32, tag="lnC")   # m*m -> lnv -> mr
    nc.vector.tensor_tensor(rowC[:], rowA[:], rowA[:], ALU.mult)
    nc.vector.tensor_tensor(rowB[:], rowB[:], rowC[:], ALU.subtract)
    nc.scalar.activation(rowC[:], rowB[:], AF.Ln, bias=epsc[0:1, 0:1])
    rowR = sb.tile([1, S], F32R, tag="lnR")  # rstd; sole writer is ACT (f32r)
    nc.scalar.activation(rowR[:], rowC[:], AF.Exp, scale=-0.5)
    nc.vector.tensor_tensor(rowC[:], rowA[:], rowR[:].bitcast(F32),
                            ALU.mult)  # m*rstd
    rstd_r = rowR[:]
    # rows2 = [c ; ones] for the K=2 C' broadcast matmul
    rows2 = sb2.tile([2, S], F32R, tag="rows2")
    nc.scalar.activation(rows2[0:1], rowC[:], AF.Copy, scale=-1.0)
    nc.sync.dma_start(rows2[1:2], ones512_d[:, :])
    for k in range(KD):
        pA = ps.tile([P, 512], F32, tag="ps")
        nc.tensor.matmul(pA[:], gbT[0:1, k * P:(k + 1) * P], rstd_r,
                         start=True, stop=True)
        pC = ps.tile([P, 512], F32, tag="ps")
        nc.tensor.matmul(pC[:], gbT[:, k * P:(k + 1) * P], rows2[:],
                         start=True, stop=True)
        t1 = sb2.tile([P, 512], F32, tag="lnt1")
        nc.vector.tensor_tensor(t1[:], hin[:, k].bitcast(F32), pA[:], ALU.mult)
        nc.vector.tensor_tensor(xout[:, k], t1[:], pC[:], ALU.add)


# ---------------------------------------------------------------------------
# vocab-projection program (launch B): logits = h_all @ tokT_shard
# ---------------------------------------------------------------------------

def build_vocab():
    nc = bacc.Bacc("TRN2", target_bir_lowering=False, debug=False,
                   num_devices=NCORES)
    hallT = nc.dram_tensor("hallT", (D, 320), F32R, kind="ExternalInput")
    tokT = nc.dram_tensor("tokT", (D, VSHP), F32R, kind="ExternalInput")
    lm = nc.dram_tensor("lm", (320, VSHP), F32, kind="ExternalOutput")
    with tile.TileContext(nc) as tc:
        with tc.tile_pool(name="sb", bufs=1) as sb, \
             tc.tile_pool(name="sb2", bufs=3) as sb2, \
             tc.tile_pool(name="ps", bufs=8, space="PSUM") as ps:
            hall = sb.tile([P, KD, 320], F32R, tag="hall")
            nc.sync.dma_start(hall[:],
                              hallT.rearrange("(ko p) t -> p ko t", p=P))
            for nv in range(VSHP // 512):
                tc_sb = sb2.tile([P, KD, 512], F32R, tag="tokc")
                nc.sync.dma_start(tc_sb[:], tokT[:, nv * 512:(nv + 1) * 512]
                                  .rearrange("(ko p) n -> p ko n", p=P))
                for mt in range(3):
                    mw = 128 if mt < 2 else 64
                    pp = ps.tile([P, 512], F32, tag="ps")
                    for k in range(KD):
                        nc.tensor.matmul(pp[:mw],
                                         hall[:, k, mt * P:mt * P + mw],
                                         tc_sb[:, k], start=(k == 0),
                                         stop=(k == KD - 1))
                    ot = sb2.tile([P, 512], F32, tag="ot")
                    nc.scalar.activation(ot[:mw], pp[:mw], AF.Copy)
                    nc.sync.dma_start(lm[mt * P:mt * P + mw,
                                         nv * 512:(nv + 1) * 512], ot[:mw])
    nc.compile()
    return nc


def build_noop():
    """Trivial program used as a launch-overhead probe for timing."""
    nc = bacc.Bacc("TRN2", target_bir_lowering=False, debug=False,
                   num_devices=NCORES)
    x = nc.dram_tensor("x", (P, P), F32, kind="ExternalInput")
    y = nc.dram_tensor("y", (P, P), F32, kind="ExternalOutput")
    with tile.TileContext(nc) as tc:
        with tc.tile_pool(name="sb", bufs=1) as sb:
            t = sb.tile([P, P], F32, tag="t")
            nc.sync.dma_start(t[:], x[:, :])
            nc.sync.dma_start(y[:, :], t[:])
    nc.compile()
    return nc


# ---------------------------------------------------------------------------
# cached PJRT runner (compile once, reuse executable + device inputs)
# ---------------------------------------------------------------------------

class Runner:
    def __init__(self, nc, n_cores=NCORES):
        import jax
        from jax.sharding import Mesh, PartitionSpec
        from jax.experimental.shard_map import shard_map
        bass2jax.install_neuronx_cc_hook()
        self.nc = nc
        self.n_cores = n_cores
        partition_name = (nc.partition_id_tensor.name
                          if nc.partition_id_tensor else None)
        in_names, out_names, out_avals, zero_shapes = [], [], [], []
        for alloc in nc.m.functions[0].allocations:
            if not isinstance(alloc, mybir.MemoryLocationSet):
                continue
            name = alloc.memorylocations[0].name
            if alloc.kind == "ExternalInput":
                if name != partition_name:
                    in_names.append(name)
            elif alloc.kind == "ExternalOutput":
                shape = tuple(alloc.tensor_shape)
                dtype = mybir.dt.np(alloc.dtype)
                out_names.append(name)
                out_avals.append(jax.core.ShapedArray(shape, dtype))
                zero_shapes.append((shape, dtype))
        self.n_params = len(in_names)
        self.in_names = list(in_names)
        self.out_names = out_names
        self.out_avals = out_avals
        self.zero_shapes = zero_shapes
        all_in = in_names + out_names
        if partition_name is not None:
            all_in = all_in + [partition_name]

        def _body(*args):
            operands = list(args)
            if partition_name is not None:
                operands.append(bass2jax.partition_id_tensor())
            outs = bass2jax._bass_exec_p.bind(
                *operands,
                out_avals=tuple(out_avals),
                in_names=tuple(all_in),
                out_names=tuple(out_names),
                lowering_input_output_aliases=(),
                sim_require_finite=True,
                sim_require_nnan=True,
                nc=nc,
            )
            return tuple(outs)

        devices = jax.devices()[:n_cores]
        self.mesh = Mesh(np.asarray(devices), ("core",))
        n_outs = len(out_names)
        donate = tuple(range(self.n_params, self.n_params + n_outs))
        self.fn = jax.jit(
            shard_map(_body, mesh=self.mesh,
                      in_specs=(PartitionSpec("core"),) * (self.n_params + n_outs),
                      out_specs=(PartitionSpec("core"),) * n_outs,
                      check_rep=False),
            donate_argnums=donate, keep_unused=True)

    def put_inputs(self, in_maps):
        import jax
        from jax.sharding import NamedSharding, PartitionSpec
        sh = NamedSharding(self.mesh, PartitionSpec("core"))
        out = []
        for name in self.in_names:
            a = np.concatenate([np.asarray(m[name]) for m in in_maps], axis=0)
            out.append(jax.device_put(a, sh))
        return out

    def zeros(self):
        import jax.numpy as jnp
        from jax.sharding import NamedSharding, PartitionSpec
        sh = NamedSharding(self.mesh, PartitionSpec("core"))
        return [jnp.zeros((self.n_cores * s[0], *s[1:]), d, device=sh)
                for (s, d) in self.zero_shapes]

    def run(self, dev_in):
        import jax
        outs = self.fn(*dev_in, *self.zeros())
        jax.block_until_ready(outs)
        return outs

    def split(self, out_arrs):
        res = []
        for c in range(self.n_cores):
            res.append({name: np.asarray(out_arrs[i])
                        .reshape(self.n_cores, *self.out_avals[i].shape)[c]
                        for i, name in enumerate(self.out_names)})
        return res

    def __call__(self, in_maps):
        return self.split(self.run(self.put_inputs(in_maps)))


# ---------------------------------------------------------------------------
# host-side preparation + execution
# ---------------------------------------------------------------------------

_CACHE = {}


def _prep_shared(params, nl=NL):
    """Build the shared (non-per-core) input arrays from params."""
    Lp = params['layers']
    g = {}
    tok = np.ascontiguousarray(np.asarray(params['tok'], dtype=np.float32))
    g['tok'] = tok
    g['pos'] = np.ascontiguousarray(np.asarray(params['pos'], np.float32)[:S])
    g['seg'] = np.ascontiguousarray(np.asarray(params['seg'], np.float32))
    eg = np.asarray(params['emb_g'], np.float32)
    eb = np.asarray(params['emb_b'], np.float32)
    g['emb_gb'] = np.ascontiguousarray(
        np.stack([eg.reshape(KD, P).T, eb.reshape(KD, P).T], axis=2))
    g['ident'] = round_fp32r(np.eye(P, dtype=np.float32))
    g['ones512'] = round_fp32r(np.ones((1, 512), np.float32))
    g['onescol'] = round_fp32r(np.ones((P, 1), np.float32))
    g['epsc'] = np.full((P, 1), 1e-5, np.float32)

    Wq = np.asarray(Lp['Wq'], np.float32)[:nl]
    Wk = np.asarray(Lp['Wk'], np.float32)[:nl]
    Wv = np.asarray(Lp['Wv'], np.float32)[:nl]
    g['Wqkv'] = round_fp32r(np.concatenate([Wq, Wk, Wv], axis=2))
    bq = np.asarray(Lp['bq'], np.float32)[:nl].reshape(nl, KD, P)
    bk = np.asarray(Lp['bk'], np.float32)[:nl].reshape(nl, KD, P)
    g['bqk'] = np.ascontiguousarray(
        np.concatenate([bq, bk], axis=1).transpose(0, 2, 1))
    g['ln1_gb'] = round_fp32r(np.stack(
        [np.asarray(Lp['ln1_g'], np.float32)[:nl],
         np.asarray(Lp['ln1_b'], np.float32)[:nl]], axis=1))
    g['ln2_gb'] = round_fp32r(np.stack(
        [np.asarray(Lp['ln2_g'], np.float32)[:nl],
         np.asarray(Lp['ln2_b'], np.float32)[:nl]], axis=1))
    g['brow'] = round_fp32r(np.concatenate(
        [np.asarray(Lp['bv'], np.float32)[:nl],
         np.asarray(Lp['bo'], np.float32)[:nl],
         np.asarray(Lp['b2'], np.float32)[:nl]], axis=1)[:, None, :])
    g['Wo'] = round_fp32r(np.asarray(Lp['Wo'], np.float32)[:nl])
    g['W1'] = round_fp32r(np.asarray(Lp['W1'], np.float32)[:nl])
    g['b1'] = np.ascontiguousarray(
        np.asarray(Lp['b1'], np.float32)[:nl].reshape(nl, KF, P)
        .transpose(0, 2, 1))
    g['W2'] = round_fp32r(np.asarray(Lp['W2'], np.float32)[:nl])
    g['pool_W'] = round_fp32r(np.asarray(params['pool_W'], np.float32))
    g['pool_b'] = np.ascontiguousarray(
        np.asarray(params['pool_b'], np.float32).reshape(KD, P).T)
    g['cls_W'] = round_fp32r(np.asarray(params['cls_W'], np.float32))
    g['cls_b'] = np.asarray(params['cls_b'], np.float32).reshape(2, 1)
    g['lin_W'] = round_fp32r(np.asarray(params['lin_W'], np.float32))
    g['lin_b'] = np.ascontiguousarray(
        np.asarray(params['lin_b'], np.float32).reshape(KD, P).T)
    return g


def make_in_maps(ids, segs, mp, g):
    in_maps = []
    for c in range(NCORES):
        im = dict(g)
        cid = ids[c * BL:(c + 1) * BL]
        cseg = segs[c * BL:(c + 1) * BL]
        cmp = mp[c * BL:(c + 1) * BL]
        im['ids_idx'] = np.ascontiguousarray(cid.reshape(8, P).T)
        im['seg_idx'] = np.ascontiguousarray(cseg.reshape(8, P).T)
        mpg = np.concatenate([cmp[0], cmp[1] + S]).astype(np.int64)
        ssel = np.zeros((P, 8, 40), np.float32)
        for j, t in enumerate(mpg):
            ssel[t % P, t // P, j] = 1.0
        im['ssel'] = ssel
        m01 = (cid != 0).astype(np.float32)
        im['mask01'] = np.ascontiguousarray(
            m01.reshape(BL, 4, P).transpose(2, 0, 1))
        in_maps.append(im)
    return in_maps


def kernel(input_ids, segment_ids, masked_pos, params):
    ids = np.asarray(input_ids).astype(np.int32)
    segs = np.asarray(segment_ids).astype(np.int32)
    mp = np.asarray(masked_pos).astype(np.int32)
    B = ids.shape[0]
    assert B == NCORES * BL

    if 'main_r' not in _CACHE:
        _CACHE['main_r'] = Runner(build_main())
    if 'vocab_r' not in _CACHE:
        _CACHE['vocab_r'] = Runner(build_vocab())

    g = _prep_shared(params)
    in_maps = make_in_maps(ids, segs, mp, g)

    results = _CACHE['main_r'](in_maps)
    hm = np.stack([results[c]['hm_out'] for c in range(NCORES)])
    clsf = np.concatenate([results[c]['clsf_out'].T
                           for c in range(NCORES)])

    # vocab projection launch (vocab-sharded over the 8 cores)
    hallT = round_fp32r(np.concatenate(list(hm), axis=1))  # [768, 320]
    in_maps2 = []
    for c in range(NCORES):
        tksh = np.zeros((D, VSHP), np.float32)
        tksh[:, :VSH] = g['tok'][c * VSH:(c + 1) * VSH, :].T
        in_maps2.append({'hallT': hallT, 'tokT': round_fp32r(tksh)})
    results2 = _CACHE['vocab_r'](in_maps2)
    lm = np.concatenate([results2[c]['lm'][:, :VSH]
                         for c in range(NCORES)], axis=1)
    logits_lm = lm.reshape(B, NPRED, V)
    return logits_lm, clsf
